# revision 1
# baseline (speedup 1.0000x reference)
"""2-layer GCN encoder on two graphs, distributed over 8 Trainium2 NeuronCores.

Strategy (v2)
-------------
Graph a -> cores 0-3, graph b -> cores 4-7. Each core owns 12,500 destination
nodes (original-id contiguous chunk) packed into 100 bins of <=128 dst slots,
balanced so every (bin, src-half) in-edge load fits 1024 slots (8 blocks of
128). Self-loops are NOT materialized as edges: the self term is added from a
host-prescaled transposed own-chunk block (dinv^2 * x_own)^T resident in SBUF.

Gathers are batched: ONE dma_gather per (group of 2 bins, half) = 2048 rows of
256 B (bf16), so SWDGE descriptor-gen fixed cost (994 ns/call) amortizes over
2048 descriptors. Tables are bf16, aggregation matmuls run bf16 (1 cyc/col vs
4 for fp32), psum accumulates fp32.

Per 128-edge block the core builds a [128 e x 128 dst] norm-scaled selection
matrix on the vector engine (is_equal*norm against an iota row) and
accumulates M^T @ S -> psum[feat, dst]. Two bins form a 256-column group that
flows through the dense chain W1 -> (+b1) relu -> W2 on chip (all bf16 in,
fp32 psum). The same compiled program serves both GCN layers:
  launch A: table = x  (bf16), weights (W1, b1, W2)       -> g
  launch B: table = g  (bf16), weights ([I|0], b2, [I;0]) -> z = relu(agg+b2)
using A_hat (x W) == (A_hat x) W so the sparse part always runs at 128
features. Host does packing / index prep / unpermute only.
"""

import os
import numpy as np

os.environ.setdefault("JAX_COMPILATION_CACHE_DIR", "/tmp/jax_cache")

import jax  # noqa: E402

try:
    jax.config.update("jax_compilation_cache_dir", "/tmp/jax_cache")
    jax.config.update("jax_persistent_cache_min_compile_time_secs", 0.0)
except Exception:
    pass

import ml_dtypes  # noqa: E402
import concourse.bacc as bacc  # noqa: E402
import concourse.tile as tile  # noqa: E402
import concourse.mybir as mybir  # noqa: E402
from concourse.bass_utils import run_bass_kernel_spmd  # noqa: E402

# ---- static problem geometry (hardcoded per contract) ----
N_NODES = 50000
D_IN = 128
D_HID = 256
HALF = 25000

N_CORES = 8
CORES_PER_GRAPH = 4
NPC = N_NODES // CORES_PER_GRAPH  # 12500 nodes per core

R = 100                    # bins per core
DTILE = 128                # dst slots per bin
NBLK_H = 8                 # 128-edge blocks per (bin, half)
CAP_H = NBLK_H * 128       # 1024 edge slots per (bin, half)
GROUPS = R // 2            # 50 dense groups of 2 bins (256 dst slots)
NCALLS = GROUPS * 2        # one dma_gather per (group, half): 2048 idx
CALL_IDX = 2 * CAP_H       # 2048 rows per gather call
IDXCOLS = CALL_IDX // 16   # 128 int16 cols per call
NBLOCKS = R * 2 * NBLK_H   # 1600 S-blocks per core
NSLOTS = R * DTILE         # 12800 dst slots per core
MBUFS = 6

BF16 = ml_dtypes.bfloat16

_prog = None


def _build_program():
    nc = bacc.Bacc("TRN2", target_bir_lowering=False, num_swdge_queues=4)
    f32 = mybir.dt.float32
    bf16 = mybir.dt.bfloat16
    tbl = nc.declare_dram_parameter("tbl", [N_NODES, D_IN], bf16, isOutput=False)
    idx = nc.declare_dram_parameter("idx", [16, NCALLS * IDXCOLS], f32, isOutput=False)
    pat = nc.declare_dram_parameter("pat", [16, 128], f32, isOutput=False)
    meta = nc.declare_dram_parameter("meta", [128, 2 * NBLOCKS], f32, isOutput=False)  # interleaved (dstrel, norm) per block
    iota = nc.declare_dram_parameter("iota", [128, DTILE], bf16, isOutput=False)
    selfp = nc.declare_dram_parameter("selfp", [128, NSLOTS], bf16, isOutput=False)
    w1 = nc.declare_dram_parameter("w1", [128, D_HID], bf16, isOutput=False)
    w2 = nc.declare_dram_parameter("w2", [D_HID, 128], bf16, isOutput=False)
    b1c = nc.declare_dram_parameter("b1c", [128, 2], f32, isOutput=False)
    gout = nc.declare_dram_parameter("gout", [GROUPS, 128, 256], bf16, isOutput=True)

    with tile.TileContext(nc) as tc:
        with (
            tc.tile_pool(name="res", bufs=1) as res,
            tc.tile_pool(name="mpool", bufs=MBUFS) as mp,
            tc.tile_pool(name="spool", bufs=4) as sp,
            tc.tile_pool(name="ssb", bufs=2) as ssb,
            tc.tile_pool(name="hsb", bufs=2) as hsb,
            tc.tile_pool(name="gsb", bufs=2) as gsbp,
            tc.tile_pool(name="psps", bufs=3, space="PSUM") as psps,
            tc.tile_pool(name="psh", bufs=2, space="PSUM") as psh,
            tc.tile_pool(name="psg", bufs=2, space="PSUM") as psg,
            tc.tile_pool(name="psi", bufs=1, space="PSUM") as psi_pool,
        ):
            pat_t = res.tile([16, 128], f32)
            nc.sync.dma_start(pat_t[:], pat[:, :])
            NIH = NCALLS * IDXCOLS // 5
            idx_raws = [res.tile([16, NIH], f32, name=f"idxraw{k}") for k in range(5)]
            for k in range(2):
                nc.sync.dma_start(idx_raws[k][:], idx[:, k * NIH:(k + 1) * NIH])
            # replicate the 16-partition wrapped index stream to 128 partitions
            # on the PE (out[p,c] = idx_raw[p%16,c]) instead of shipping the
            # 8x-replicated tile through the DMA engines.
            NIREP = (NCALLS * IDXCOLS) // 512
            idx_ts = [res.tile([128, 512], mybir.dt.int16, name=f"idxr{j}")
                      for j in range(NIREP)]

            PER_RAW = NIH // 512

            def rep_chunk(j):
                if j % PER_RAW == 0 and j // PER_RAW + 2 < 5:
                    k = j // PER_RAW + 2
                    nc.sync.dma_start(idx_raws[k][:], idx[:, k * NIH:(k + 1) * NIH])
                psi = psi_pool.tile([128, 512], f32, tag="psi")
                nc.tensor.matmul(out=psi[:], lhsT=pat_t[:],
                                 rhs=idx_raws[j // PER_RAW][:, (j % PER_RAW) * 512:(j % PER_RAW + 1) * 512],
                                 start=True, stop=True)
                nc.scalar.activation(idx_ts[j][:], psi[:],
                                     mybir.ActivationFunctionType.Copy)

            for j in range(3):
                rep_chunk(j)
            meta_t = res.tile([128, 2 * NBLOCKS], f32)
            nc.sync.dma_start(meta_t[:], meta[:, :])
            iota_t = res.tile([128, DTILE], bf16)
            nc.sync.dma_start(iota_t[:], iota[:, :])
            self_t = res.tile([128, NSLOTS], bf16)
            nc.sync.dma_start(self_t[:], selfp[:, :])
            w1t = res.tile([128, D_HID], bf16)
            nc.sync.dma_start(w1t[:], w1[:, :])
            w2t = res.tile([128, D_HID], bf16)
            nc.sync.dma_start(w2t[:, 0:128], w2[0:128, :])
            nc.sync.dma_start(w2t[:, 128:256], w2[128:256, :])
            b1t = res.tile([128, 2], f32)
            nc.sync.dma_start(b1t[:], b1c[:, :])

            iota_ap = iota_t[:]

            for q in range(GROUPS):
                jnext = q // 2 + 3
                if q % 2 == 0 and jnext < NIREP:
                    rep_chunk(jnext)
                ms = []
                for h in range(2):
                    call = q * 2 + h
                    m = mp.tile([128, CALL_IDX], bf16, tag="m")
                    nc.gpsimd.dma_gather(
                        out_ap=m[:].rearrange("p (b e) -> p b e", e=D_IN),
                        in_ap=tbl[h * HALF:(h + 1) * HALF, :],
                        idxs_ap=idx_ts[call // 4][:, (call % 4) * IDXCOLS:(call % 4 + 1) * IDXCOLS],
                        num_idxs=CALL_IDX,
                        num_idxs_reg=CALL_IDX,
                        elem_size=D_IN,
                        single_packet=False,
                        queue_num=call % 4,
                    )
                    ms.append(m)
                s_sb = ssb.tile([128, 256], bf16, tag="s_sb")
                for bb in range(2):
                    r = q * 2 + bb
                    ps = psps.tile([128, DTILE], f32, tag="ps")
                    for h in range(2):
                        for b in range(NBLK_H):
                            blkcol = ((q * 2 + h) * 2 + bb) * NBLK_H + b
                            s = sp.tile([128, DTILE], bf16, tag="s")
                            nc.vector.tensor_scalar(
                                out=s[:],
                                in0=iota_ap,
                                scalar1=meta_t[:, 2 * blkcol:2 * blkcol + 1],
                                scalar2=meta_t[:, 2 * blkcol + 1:2 * blkcol + 2],
                                op0=mybir.AluOpType.is_equal,
                                op1=mybir.AluOpType.mult,
                            )
                            nc.tensor.matmul(
                                out=ps[:],
                                lhsT=ms[h][:, (bb * NBLK_H + b) * 128:(bb * NBLK_H + b + 1) * 128],
                                rhs=s[:],
                                start=(h == 0 and b == 0),
                                stop=(h == 1 and b == NBLK_H - 1),
                            )
                    nc.vector.tensor_tensor(
                        out=s_sb[:, bb * DTILE:(bb + 1) * DTILE],
                        in0=ps[:],
                        in1=self_t[:, r * DTILE:(r + 1) * DTILE],
                        op=mybir.AluOpType.add,
                    )

                h1ps = psh.tile([128, 512], f32, tag="h1ps")
                nc.tensor.matmul(out=h1ps[:, 0:256], lhsT=w1t[:, 0:128], rhs=s_sb[:], start=True, stop=True)
                nc.tensor.matmul(out=h1ps[:, 256:512], lhsT=w1t[:, 128:256], rhs=s_sb[:], start=True, stop=True)
                h1 = hsb.tile([128, 512], bf16, tag="h1")
                nc.scalar.activation(h1[:, 0:256], h1ps[:, 0:256], mybir.ActivationFunctionType.Relu, bias=b1t[:, 0:1])
                nc.scalar.activation(h1[:, 256:512], h1ps[:, 256:512], mybir.ActivationFunctionType.Relu, bias=b1t[:, 1:2])
                gps = psg.tile([128, 256], f32, tag="gps")
                nc.tensor.matmul(out=gps[:], lhsT=w2t[:, 0:128], rhs=h1[:, 0:256], start=True, stop=False)
                nc.tensor.matmul(out=gps[:], lhsT=w2t[:, 128:256], rhs=h1[:, 256:512], start=False, stop=True)
                gsb = gsbp.tile([128, 256], bf16, tag="gsb")
                nc.scalar.activation(gsb[:], gps[:], mybir.ActivationFunctionType.Copy)
                nc.sync.dma_start(gout[q], gsb[:])

    nc.compile()
    return nc


def _get_program():
    global _prog
    if _prog is None:
        _prog = _build_program()
    return _prog


def _pack_core(deg2):
    """Greedy balance 12,500 nodes into R bins: <=DTILE nodes/bin and both
    per-half edge loads <= CAP_H. deg2: [NPC, 2]."""
    tot = deg2[:, 0] + deg2[:, 1]
    order = np.argsort(-tot, kind="stable")
    binload = np.zeros((R, 2), np.int64)
    bincnt = np.zeros(R, np.int64)
    bin_of = np.full(NPC, -1, np.int32)
    for v in order:
        d0, d1 = deg2[v]
        ok = (bincnt < DTILE) & (binload[:, 0] + d0 <= CAP_H) & (binload[:, 1] + d1 <= CAP_H)
        if not ok.any():
            return None
        score = np.maximum(binload[:, 0] + d0, binload[:, 1] + d1)
        score = np.where(ok, score, 1 << 30)
        b = int(np.argmin(score))
        bin_of[v] = b
        bincnt[b] += 1
        binload[b, 0] += d0
        binload[b, 1] += d1
    return bin_of


def _preprocess_graph(edge):
    """Per graph: per-core packing + slot assembly. Returns (cores, dinv)."""
    src = np.asarray(edge[0], np.int64)
    dst = np.asarray(edge[1], np.int64)
    deg = np.bincount(dst, minlength=N_NODES).astype(np.float32)
    dinv = (1.0 / np.sqrt(deg + np.float32(1.0))).astype(np.float32)
    anorm = (dinv[src] * dinv[dst]).astype(np.float32)
    ehalf = (src >= HALF).astype(np.int64)

    cores = []
    for c in range(CORES_PER_GRAPH):
        lo, hi = c * NPC, (c + 1) * NPC
        emask = (dst >= lo) & (dst < hi)
        es = src[emask]
        ed = dst[emask] - lo
        en = anorm[emask]
        eh = ehalf[emask]

        deg2 = np.zeros((NPC, 2), np.int64)
        np.add.at(deg2, (ed, eh), 1)
        bin_of = _pack_core(deg2)
        assert bin_of is not None, "bin packing failed"

        # position of each node within its bin
        order2 = np.lexsort((np.arange(NPC), bin_of))
        pos_in_bin = np.empty(NPC, np.int64)
        binstart = np.zeros(R + 1, np.int64)
        np.cumsum(np.bincount(bin_of, minlength=R), out=binstart[1:])
        pos_in_bin[order2] = np.arange(NPC) - binstart[bin_of[order2]]

        # column map: group q, col -> global node id (or -1); self slot rows
        cols_map = np.full((GROUPS, 256), -1, np.int64)
        q_of = bin_of // 2
        col_of = (bin_of % 2) * DTILE + pos_in_bin
        cols_map[q_of, col_of] = np.arange(lo, hi)
        self_rows = np.full(NSLOTS, -1, np.int64)
        self_rows[bin_of * DTILE + pos_in_bin] = np.arange(lo, hi)

        # --- edge slot assembly: stream k = ((q*2 + h)*2 + bb), cap 1024 ---
        k = (q_of[ed] * 2 + eh) * 2 + (bin_of[ed] % 2)
        okey = np.lexsort((np.arange(len(es)), k))
        ksorted = k[okey]
        counts = np.bincount(ksorted, minlength=R * 2)
        assert counts.max() <= CAP_H
        starts = np.zeros(R * 2 + 1, np.int64)
        np.cumsum(counts, out=starts[1:])
        within = np.arange(len(es)) - starts[ksorted]
        slot = ksorted * CAP_H + within

        # pad slots gather row 0 with dstrel=-1 (S column contribution 0)
        total = NCALLS * CALL_IDX
        idx_slots = np.zeros(total, np.int64)
        dst_slots = np.full(total, -1.0, np.float32)
        nrm_slots = np.zeros(total, np.float32)
        idx_slots[slot] = es[okey] - eh[okey] * HALF
        dst_slots[slot] = pos_in_bin[ed[okey]].astype(np.float32)
        nrm_slots[slot] = en[okey]

        a = idx_slots.reshape(NCALLS, IDXCOLS, 16)
        idx16 = np.ascontiguousarray(
            np.transpose(a, (2, 0, 1)).reshape(16, NCALLS * IDXCOLS)).astype(np.float32)
        pair = np.stack([dst_slots.reshape(NBLOCKS, 128),
                         nrm_slots.reshape(NBLOCKS, 128)], axis=1)  # [NBLOCKS, 2, 128]
        meta = np.ascontiguousarray(pair.reshape(NBLOCKS * 2, 128).T).astype(np.float32)

        cores.append({
            "idx": idx16, "meta": meta, "cols_map": cols_map,
            "self_rows": self_rows, "lo": lo,
        })
    return cores, dinv


def _self_block(core, tbl_f32, dinv):
    """[128, NSLOTS] bf16: column (bin*128+pos) = dinv^2[v] * tbl[v]."""
    sr = core["self_rows"]
    valid = sr >= 0
    blk = np.zeros((NSLOTS, D_IN), np.float32)
    v = sr[valid]
    blk[valid] = tbl_f32[v] * (dinv[v] * dinv[v])[:, None]
    return np.ascontiguousarray(blk.T).astype(BF16)


def _assemble(results, cores_list):
    """Gather per-core gout into full [N, 128] fp32 arrays for each graph."""
    outs = []
    for g, cores in enumerate(cores_list):
        full = np.zeros((N_NODES, D_IN), np.float32)
        for c in range(CORES_PER_GRAPH):
            go = np.asarray(results[g * CORES_PER_GRAPH + c]["gout"], dtype=np.float32)
            cm = cores[c]["cols_map"]
            for q in range(GROUPS):
                valid = cm[q] >= 0
                full[cm[q][valid]] = go[q][:, valid].T
        outs.append(full)
    return outs


def _spot_check(full, tbl, edge, dinv, post, n_samples=24, tol=5e-2):
    """Verify a few random nodes of a launch output on host (numpy)."""
    src = np.asarray(edge[0], np.int64)
    dst = np.asarray(edge[1], np.int64)
    rng = np.random.default_rng(12345)
    nodes = rng.integers(0, N_NODES, size=n_samples)
    for v in nodes:
        ine = np.where(dst == v)[0]
        s = (dinv[src[ine]] * dinv[v])[:, None] * tbl[src[ine]]
        s = s.sum(axis=0, dtype=np.float64) + np.float64(dinv[v]) ** 2 * tbl[v]
        exp = post(s)
        got = full[v]
        scale = max(np.abs(exp).max(), 1e-3)
        if np.abs(got - exp).max() / scale > tol:
            return False
    return True


LAUNCH_WALL = []
IOTA = np.ascontiguousarray(
    np.broadcast_to(np.arange(DTILE, dtype=np.float32), (128, DTILE))).astype(BF16)
PAT = (np.arange(128)[None, :] % 16 == np.arange(16)[:, None]).astype(np.float32)


def kernel(x_a, edge_a, x_b, edge_b, W1, b1, W2, b2):
    x_a = np.ascontiguousarray(np.asarray(x_a, np.float32))
    x_b = np.ascontiguousarray(np.asarray(x_b, np.float32))
    W1 = np.asarray(W1, np.float32)
    b1 = np.asarray(b1, np.float32)
    W2 = np.asarray(W2, np.float32)
    b2 = np.asarray(b2, np.float32)

    nc = _get_program()
    cores_a, dinv_a = _preprocess_graph(np.asarray(edge_a))
    cores_b, dinv_b = _preprocess_graph(np.asarray(edge_b))

    b1c = np.stack([b1[0:128], b1[128:256]], axis=1).astype(np.float32)
    eye = np.eye(128, dtype=np.float32)
    w1_id = np.concatenate([eye, np.zeros((128, 128), np.float32)], axis=1).astype(BF16)
    w2_id = np.concatenate([eye, np.zeros((128, 128), np.float32)], axis=0).astype(BF16)
    b1c_id = np.stack([b2, np.zeros(128, np.float32)], axis=1).astype(np.float32)
    w1_b = W1.astype(BF16)
    w2_b = W2.astype(BF16)

    def maps(tbl_a, tbl_b, w1m, w2m, b1m):
        tba = tbl_a.astype(BF16)
        tbb = tbl_b.astype(BF16)
        ms = []
        for tb, tf, cores, dinv in ((tba, tbl_a, cores_a, dinv_a),
                                    (tbb, tbl_b, cores_b, dinv_b)):
            for c in range(CORES_PER_GRAPH):
                ms.append({
                    "tbl": tb,
                    "idx": cores[c]["idx"],
                    "meta": cores[c]["meta"],
                    "iota": IOTA,
                    "pat": PAT,
                    "selfp": _self_block(cores[c], tf, dinv),
                    "w1": w1m, "w2": w2m, "b1c": b1m,
                })
        return ms

    core_ids = list(range(N_CORES))

    def run(in_maps):
        import time as _t
        last = None
        for attempt in range(4):
            try:
                t0 = _t.time()
                res = run_bass_kernel_spmd(nc, in_maps, core_ids)
                LAUNCH_WALL.append(_t.time() - t0)
                return res
            except Exception as e:  # wedged core recovers on retry
                last = e
                _t.sleep(5)
        raise last

    def post_a(s):
        return np.maximum(s @ W1.astype(np.float64) + b1, 0.0) @ W2.astype(np.float64)

    def post_b(s):
        return np.maximum(s + b2, 0.0)

    # run each launch until the host spot-check passes (guards against rare
    # silent device-side corruption)
    for attempt in range(4):
        resA = run(maps(x_a, x_b, w1_b, w2_b, b1c))
        g_a, g_b = _assemble(resA.results, (cores_a, cores_b))
        if (_spot_check(g_a, x_a, edge_a, dinv_a, post_a)
                and _spot_check(g_b, x_b, edge_b, dinv_b, post_a)):
            break
    for attempt in range(4):
        resB = run(maps(g_a, g_b, w1_id, w2_id, b1c_id))
        z_a, z_b = _assemble(resB.results, (cores_a, cores_b))
        if (_spot_check(z_a, g_a, edge_a, dinv_a, post_b)
                and _spot_check(z_b, g_b, edge_b, dinv_b, post_b)):
            break
    return (z_a, z_b)



# revision 22
# speedup vs baseline: 1.0417x; 1.0417x over previous
"""2-layer GCN encoder on two graphs, distributed over 8 Trainium2 NeuronCores.

Strategy (v3): dual-engine gather
---------------------------------
Graph a -> cores 0-3, graph b -> cores 4-7. Each core owns 12,500 destination
nodes packed into R=104 bins of <=128 dst slots. Per-edge source rows are
fetched by TWO engines in parallel:

  - P-path (GPSIMD/Pool): sources in [0, 25000) are gathered by ap_gather
    from an SBUF-resident feature-major table packed as int32 node-PAIRS
    (ftab32[p, m] = bf16x2(x[2m, p], x[2m+1, p])). Edges split into E (even
    src) and O (odd src) classes; idx = src >> 1. The gathered M^T columns
    are transposed back to edge-major M-tiles on the PE (stride-2 bf16
    parity views -> identity transpose matmul -> psum) and copied to SBUF by
    the Activation engine.
  - D-path (DMA/SWDGE): sources in [17232, 50000) use dma_gather from the
    node-major HBM table (window start 17232 keeps idx within int16).
    Sources in the overlap [17232, 25000) are "flex": normally E/O, demoted
    to D when bin caps require.

Per bin: 4 E-blocks + 4 O-blocks + 8 D-blocks of 128 edge slots. A [128 e x
128 d] 0/norm selection matrix per block (DVE iota/is_equal) accumulates
M^T @ S into psum[feat, dst]; self-loops come from a host-prescaled
transposed block (dinv^2 * x)^T. Two bins form a 256-col group that flows
through W1 -> relu -> W2 on chip. The same compiled program serves both GCN
layers (A-hat (x W) == (A-hat x) W):
  launch A: table = x  -> g = relu(A x W1 + b1) W2
  launch B: table = g, identity weights -> z = relu(A g + b2)
"""

import os
import numpy as np

os.environ.setdefault("JAX_COMPILATION_CACHE_DIR", "/tmp/jax_cache")

import jax  # noqa: E402

try:
    jax.config.update("jax_compilation_cache_dir", "/tmp/jax_cache")
    jax.config.update("jax_persistent_cache_min_compile_time_secs", 0.0)
except Exception:
    pass

import ml_dtypes  # noqa: E402
import concourse.bacc as bacc  # noqa: E402
import concourse.tile as tile  # noqa: E402
import concourse.mybir as mybir  # noqa: E402
from concourse.bass_utils import run_bass_kernel_spmd  # noqa: E402

# ---- static problem geometry ----
N_NODES = 50000
D_IN = 128
D_HID = 256
HALF = 25000
DWIN = 17232              # D-gather window start: 50000-DWIN = 32768 (int16)

N_CORES = 8
CORES_PER_GRAPH = 4
NPC = N_NODES // CORES_PER_GRAPH  # 12500 dst nodes per core

R = 104                   # bins per core (divisible by 8)
DTILE = 128
QUART = 12500             # sub-table node split: T1=[0,12500), T2=[12500,25000)
BLK_P = 2                 # blocks per bin per P subclass (E1,O1,E2,O2)
BLK_D = 8                 # D blocks per bin (cap 1024)
CAP_P = BLK_P * 128       # 256
CAP_D = BLK_D * 128
NBLK = 4 * BLK_P + BLK_D              # 16 blocks per bin
NBLOCKS = R * NBLK                    # 1664
NSLOTS = R * DTILE                    # 13312 dst slots
GROUPS = R // 2                       # 52 dense groups

CHUNKS = 8
BINS_PER_CHUNK = R // CHUNKS          # 13
PCALL = BINS_PER_CHUNK * 2 * CAP_P    # 6656 idx per ap_gather call (E+O)
NPAIR = HALF // 2                     # 12500 pair elements
NPSUB = QUART // 2                    # 6250 pairs per sub-table

DCALL_BLK = 44                        # D blocks per dma_gather call
DCALL = DCALL_BLK * 128               # 5632 descs
ND_BLOCKS = R * BLK_D                 # 832 D blocks per core
ND_CALLS = (ND_BLOCKS + DCALL_BLK - 1) // DCALL_BLK   # 19 (last partial)

BF16 = ml_dtypes.bfloat16

_progs = {}


def _build_program():
    nc = bacc.Bacc("TRN2", target_bir_lowering=False, num_swdge_queues=4)
    f32 = mybir.dt.float32
    bf16 = mybir.dt.bfloat16
    i16 = mybir.dt.int16
    i32 = mybir.dt.int32

    tbl = nc.declare_dram_parameter("tbl", [N_NODES, D_IN], bf16, isOutput=False)
    ftab = nc.declare_dram_parameter("ftab", [128, NPAIR], i32, isOutput=False)
    idxe = nc.declare_dram_parameter("idxe", [128, R * 2 * CAP_P // 16], i16, isOutput=False)
    idxo = nc.declare_dram_parameter("idxo", [128, R * 2 * CAP_P // 16], i16, isOutput=False)
    idxd = nc.declare_dram_parameter("idxd", [128, R * CAP_D // 16], i16, isOutput=False)
    meta = nc.declare_dram_parameter("meta", [128, 2 * NBLOCKS], f32, isOutput=False)
    iota = nc.declare_dram_parameter("iota", [128, DTILE], bf16, isOutput=False)
    ident = nc.declare_dram_parameter("ident", [128, 128], bf16, isOutput=False)
    selfp = nc.declare_dram_parameter("selfp", [128, NSLOTS], bf16, isOutput=False)
    w1 = nc.declare_dram_parameter("w1", [128, D_HID], bf16, isOutput=False)
    w2 = nc.declare_dram_parameter("w2", [D_HID, 128], bf16, isOutput=False)
    b1c = nc.declare_dram_parameter("b1c", [128, 2], f32, isOutput=False)
    gout = nc.declare_dram_parameter("gout", [GROUPS, 128, 256], bf16, isOutput=True)

    ECOLS = PCALL // 16          # idx cols per chunk per sub-table (416)
    DCOLS = DCALL // 16          # idx cols per full D call (352)
    SELF_BINS = 4                # selfp stream granularity (bins)
    META_BINS = 8                # meta stream granularity (bins)

    from contextlib import ExitStack
    with tile.TileContext(nc) as tc:
        with ExitStack() as _stk:
            res = _stk.enter_context(tc.tile_pool(name="res", bufs=1))
            mtep = _stk.enter_context(tc.tile_pool(name="mte", bufs=2))
            mtop = _stk.enter_context(tc.tile_pool(name="mto", bufs=2))
            dmp = _stk.enter_context(tc.tile_pool(name="dmp", bufs=2))
            mtilep = _stk.enter_context(tc.tile_pool(name="mtile", bufs=4))
            iep = _stk.enter_context(tc.tile_pool(name="ie", bufs=2))
            iop = _stk.enter_context(tc.tile_pool(name="io", bufs=2))
            idp = _stk.enter_context(tc.tile_pool(name="idp", bufs=2))
            sfp = _stk.enter_context(tc.tile_pool(name="sfp", bufs=2))
            mtp_pool = _stk.enter_context(tc.tile_pool(name="mtp", bufs=2))
            sp = _stk.enter_context(tc.tile_pool(name="spool", bufs=8))
            ssb = _stk.enter_context(tc.tile_pool(name="ssb", bufs=2))
            hsb = _stk.enter_context(tc.tile_pool(name="hsb", bufs=2))
            gsbp = _stk.enter_context(tc.tile_pool(name="gsb", bufs=2))
            psps = _stk.enter_context(tc.tile_pool(name="psps", bufs=4, space="PSUM"))
            pstp = _stk.enter_context(tc.tile_pool(name="pst", bufs=2, space="PSUM"))
            psh = _stk.enter_context(tc.tile_pool(name="psh", bufs=1, space="PSUM"))
            psg = _stk.enter_context(tc.tile_pool(name="psg", bufs=1, space="PSUM"))
            # resident small tensors
            iota_t = res.tile([128, DTILE], bf16)
            nc.sync.dma_start(iota_t[:], iota[:, :])
            id_t = res.tile([128, 128], bf16)
            nc.sync.dma_start(id_t[:], ident[:, :])
            w1t = res.tile([128, D_HID], bf16)
            nc.sync.dma_start(w1t[:], w1[:, :])
            w2t = res.tile([128, D_HID], bf16)
            nc.sync.dma_start(w2t[:, 0:128], w2[0:128, :])
            nc.sync.dma_start(w2t[:, 128:256], w2[128:256, :])
            b1t = res.tile([128, 2], f32)
            nc.sync.dma_start(b1t[:], b1c[:, :])
            # big resident: feature-major pair table (50 KB/partition)
            ftab_t = res.tile([128, NPAIR], i32)
            for q in range(4):
                nc.sync.dma_start(ftab_t[:, q * (NPAIR // 4):(q + 1) * (NPAIR // 4)],
                                  ftab[:, q * (NPAIR // 4):(q + 1) * (NPAIR // 4)])

            # streamed tiles state
            dm_tiles = [None] * ND_CALLS
            meta_tiles = {}
            self_tiles = {}

            def ensure_dcall(k):
                if dm_tiles[k] is not None:
                    return
                nblk = min(DCALL_BLK, ND_BLOCKS - k * DCALL_BLK)
                nidx = nblk * 128
                it = idp.tile([128, DCOLS], i16, tag="idp")
                nc.sync.dma_start(it[:, 0:nidx // 16],
                                  idxd[:, k * DCOLS:k * DCOLS + nidx // 16])
                dm = dmp.tile([128, DCALL_BLK, 128], bf16, tag="dm")
                nc.gpsimd.dma_gather(
                    out_ap=dm[:, 0:nblk, :],
                    in_ap=tbl[DWIN:N_NODES, :],
                    idxs_ap=it[:, 0:nidx // 16],
                    num_idxs=nidx,
                    num_idxs_reg=nidx,
                    elem_size=D_IN,
                    single_packet=False,
                    queue_num=k % 4,
                )
                dm_tiles[k] = dm

            # start the D-stream before the (ftab-gated) P-path gathers so
            # the DMA engines are busy from the first microsecond
            ensure_dcall(0)
            ensure_dcall(1)

            chunk_views = {}
            chunk_loads = {}

            def ensure_chunk_loads(c):
                if c in chunk_loads:
                    return chunk_loads[c]
                iet = iep.tile([128, ECOLS], i16, tag="ie")
                nc.sync.dma_start(iet[:], idxe[:, c * ECOLS:(c + 1) * ECOLS])
                iot = iop.tile([128, ECOLS], i16, tag="io")
                nc.sync.dma_start(iot[:], idxo[:, c * ECOLS:(c + 1) * ECOLS])
                chunk_loads[c] = (iet, iot)
                return chunk_loads[c]

            def ensure_chunk(c):
                if c in chunk_views:
                    return chunk_views[c]
                iet, iot = ensure_chunk_loads(c)
                mte = mtep.tile([128, PCALL], i32, tag="mte")
                nc.gpsimd.ap_gather(
                    out_ap=mte[:], in_ap=ftab_t[:, 0:NPSUB], idxs_ap=iet[:],
                    channels=128, num_elems=NPSUB, d=1, num_idxs=PCALL,
                )
                mto = mtop.tile([128, PCALL], i32, tag="mto")
                nc.gpsimd.ap_gather(
                    out_ap=mto[:], in_ap=ftab_t[:, NPSUB:NPAIR], idxs_ap=iot[:],
                    channels=128, num_elems=NPSUB, d=1, num_idxs=PCALL,
                )
                mtev = mte[:].bitcast(bf16).rearrange("p (n two) -> p n two", two=2)
                mtov = mto[:].bitcast(bf16).rearrange("p (n two) -> p n two", two=2)
                chunk_views[c] = (mtev, mtov)
                return chunk_views[c]

            mtiles = {}

            def ensure_meta(r):
                mkey = r // META_BINS
                if mkey not in meta_tiles:
                    mt_ = mtp_pool.tile([128, 2 * NBLK * META_BINS], f32,
                                        tag="meta", name="meta_t")
                    lo = mkey * META_BINS * NBLK * 2
                    hi = min(2 * NBLOCKS, lo + 2 * NBLK * META_BINS)
                    nc.sync.dma_start(mt_[:, 0:hi - lo], meta[:, lo:hi])
                    meta_tiles[mkey] = mt_
                skey = r // SELF_BINS
                if skey not in self_tiles:
                    st_ = sfp.tile([128, SELF_BINS * DTILE], bf16, tag="sf",
                                   name="self_t")
                    lo = skey * SELF_BINS * DTILE
                    nc.sync.dma_start(st_[:], selfp[:, lo:lo + SELF_BINS * DTILE])
                    self_tiles[skey] = st_

            def prep(r):
                """Transpose bin r's P-path M^T columns into an edge-major
                M-tile, and kick the gathers later bins will need. Runs 2
                bins ahead of agg() so the copy latency stays off the agg
                path. stream layout per bin: [E 256 | O 256] per sub-table;
                E = even sources (parity 0), O = odd (parity 1). mtile
                blocks: 0-1 E1, 2-3 O1, 4-5 E2, 6-7 O2."""
                ensure_dcall(min(((r + 3) * BLK_D + BLK_D - 1) // DCALL_BLK,
                                 ND_CALLS - 1))
                ensure_meta(r)
                ensure_chunk_loads(min((r + 9) // BINS_PER_CHUNK, CHUNKS - 1))
                mtev, mtov = ensure_chunk(r // BINS_PER_CHUNK)
                ensure_chunk(min((r + 6) // BINS_PER_CHUNK, CHUNKS - 1))
                bb = r % BINS_PER_CHUNK
                mtile = mtilep.tile([128, 1024], bf16, tag="mtile")
                boff = bb * 2 * CAP_P
                ps_ = pstp.tile([128, 1024], bf16, tag="pst")
                for half, mv in ((0, mtev), (1, mtov)):
                    for k in range(2 * BLK_P):
                        par = 0 if k < BLK_P else 1
                        col0 = boff + k * 128
                        nc.tensor.transpose(
                            ps_[:, (half * 4 + k) * 128:(half * 4 + k + 1) * 128],
                            mv[:, col0:col0 + 128, par],
                            id_t[:],
                        )
                nc.scalar.activation(mtile[:], ps_[:],
                                     mybir.ActivationFunctionType.Copy)
                mtiles[r] = mtile

            group_state = {}

            bin_ps = {}

            def agg(r):
                ensure_meta(r)
                meta_t = meta_tiles[r // META_BINS]
                moff = (r % META_BINS) * NBLK * 2
                mtile = mtiles.pop(r)

                ps = psps.tile([128, DTILE], f32, tag="ps")
                for blk in range(NBLK):
                    s = sp.tile([128, DTILE], bf16, tag="s")
                    nc.vector.tensor_scalar(
                        out=s[:],
                        in0=iota_t[:],
                        scalar1=meta_t[:, moff + 2 * blk:moff + 2 * blk + 1],
                        scalar2=meta_t[:, moff + 2 * blk + 1:moff + 2 * blk + 2],
                        op0=mybir.AluOpType.is_equal,
                        op1=mybir.AluOpType.mult,
                    )
                    if blk < 4 * BLK_P:
                        lhsT = mtile[:, blk * 128:(blk + 1) * 128]
                    else:
                        db = r * BLK_D + (blk - 4 * BLK_P)
                        k, kb = db // DCALL_BLK, db % DCALL_BLK
                        ensure_dcall(k)
                        lhsT = dm_tiles[k][:, kb, :]
                    nc.tensor.matmul(
                        out=ps[:], lhsT=lhsT, rhs=s[:],
                        start=(blk == 0), stop=(blk == NBLK - 1),
                    )
                bin_ps[r] = ps

            def combine(r):
                """Self-add + dense chain, deferred one bin behind agg so the
                DVE stream never stalls waiting the bin's last matmul."""
                ps = bin_ps.pop(r)
                skey = r // SELF_BINS
                if r % 2 == 0:
                    group_state["s_sb"] = ssb.tile([128, 256], bf16,
                                                   tag="s_sb", name="s_sb")
                s_sb = group_state["s_sb"]
                nc.vector.tensor_tensor(
                    out=s_sb[:, (r % 2) * DTILE:(r % 2 + 1) * DTILE],
                    in0=ps[:],
                    in1=self_tiles[skey][:, (r % SELF_BINS) * DTILE:
                                         (r % SELF_BINS + 1) * DTILE],
                    op=mybir.AluOpType.add,
                )
                if r % 2 == 1:
                    q = r // 2
                    h1ps = psh.tile([128, 512], f32, tag="h1ps")
                    nc.tensor.matmul(out=h1ps[:, 0:256], lhsT=w1t[:, 0:128],
                                     rhs=s_sb[:], start=True, stop=True)
                    nc.tensor.matmul(out=h1ps[:, 256:512], lhsT=w1t[:, 128:256],
                                     rhs=s_sb[:], start=True, stop=True)
                    h1 = hsb.tile([128, 512], bf16, tag="h1")
                    nc.scalar.activation(h1[:, 0:256], h1ps[:, 0:256],
                                         mybir.ActivationFunctionType.Relu,
                                         bias=b1t[:, 0:1])
                    nc.scalar.activation(h1[:, 256:512], h1ps[:, 256:512],
                                         mybir.ActivationFunctionType.Relu,
                                         bias=b1t[:, 1:2])
                    gps = psg.tile([128, 256], f32, tag="gps")
                    nc.tensor.matmul(out=gps[:], lhsT=w2t[:, 0:128],
                                     rhs=h1[:, 0:256], start=True, stop=False)
                    nc.tensor.matmul(out=gps[:], lhsT=w2t[:, 128:256],
                                     rhs=h1[:, 256:512], start=False, stop=True)
                    gsb = gsbp.tile([128, 256], bf16, tag="gsb")
                    nc.scalar.activation(gsb[:], gps[:],
                                         mybir.ActivationFunctionType.Copy)
                    nc.sync.dma_start(gout[q], gsb[:])

            PIPE = 3
            CDEF = 2
            for r in range(R + PIPE + CDEF):
                if r >= PIPE + CDEF:
                    combine(r - PIPE - CDEF)
                if r < R:
                    prep(r)
                if PIPE <= r < R + PIPE:
                    agg(r - PIPE)

    nc.compile()
    return nc


def _get_program():
    if "p" not in _progs:
        _progs["p"] = _build_program()
    return _progs["p"]


CAPS5 = (CAP_P, CAP_P, CAP_P, CAP_P, CAP_D)


def _pack_core(deg5):
    """Greedy-balance NPC nodes into R bins with per-class caps.
    deg5: [NPC, 5] (E1, O1, E2, O2, D) degree per node. Returns bin_of or None."""
    caps = np.array(CAPS5, np.int64)
    tot = deg5.sum(axis=1)
    order = np.argsort(-tot, kind="stable")
    binload = np.zeros((R, 5), np.int64)
    bincnt = np.zeros(R, np.int64)
    bin_of = np.full(NPC, -1, np.int32)
    for v in order:
        d = deg5[v]
        nl = binload + d
        ok = (bincnt < DTILE) & (nl <= caps).all(axis=1)
        if not ok.any():
            return None
        score = (nl.astype(np.float64) / caps).max(axis=1)
        score = np.where(ok, score, np.inf)
        b = int(np.argmin(score))
        bin_of[v] = b
        bincnt[b] += 1
        binload[b] += d
    return bin_of


def _preprocess_graph(edge):
    """Per graph: class assignment, per-core packing, slot assembly."""
    src = np.asarray(edge[0], np.int64)
    dst = np.asarray(edge[1], np.int64)
    deg = np.bincount(dst, minlength=N_NODES).astype(np.float32)
    dinv = (1.0 / np.sqrt(deg + np.float32(1.0))).astype(np.float32)
    anorm = (dinv[src] * dinv[dst]).astype(np.float32)

    cores = []
    for c in range(CORES_PER_GRAPH):
        lo, hi = c * NPC, (c + 1) * NPC
        emask = (dst >= lo) & (dst < hi)
        es = src[emask]
        ed = dst[emask] - lo
        en = anorm[emask]

        # class: 0=E1, 1=O1 (src<QUART), 2=E2, 3=O2 (QUART<=src<HALF), 4=D;
        # flex zone [DWIN, HALF) demotable from E2/O2 to D
        ecls = np.where(es >= HALF, 4,
                        np.where(es < QUART, es % 2, 2 + es % 2)).astype(np.int64)
        flex = (es >= DWIN) & (es < HALF)

        for attempt in range(8):
            deg5 = np.zeros((NPC, 5), np.int64)
            np.add.at(deg5, (ed, ecls), 1)
            bin_of = _pack_core(deg5)
            if bin_of is not None:
                break
            # demote a random slice of flex edges to D and retry
            fi = np.where(flex & (ecls != 4))[0]
            rng = np.random.default_rng(attempt)
            take = fi[rng.random(len(fi)) < 0.25]
            ecls[take] = 4
        assert bin_of is not None, "bin packing failed"

        order2 = np.lexsort((np.arange(NPC), bin_of))
        pos_in_bin = np.empty(NPC, np.int64)
        binstart = np.zeros(R + 1, np.int64)
        np.cumsum(np.bincount(bin_of, minlength=R), out=binstart[1:])
        pos_in_bin[order2] = np.arange(NPC) - binstart[bin_of[order2]]

        cols_map = np.full((GROUPS, 256), -1, np.int64)
        q_of = bin_of // 2
        col_of = (bin_of % 2) * DTILE + pos_in_bin
        cols_map[q_of, col_of] = np.arange(lo, hi)
        self_rows = np.full(NSLOTS, -1, np.int64)
        self_rows[bin_of * DTILE + pos_in_bin] = np.arange(lo, hi)

        # --- slot assembly per class stream ---
        ebin = bin_of[ed]
        epos = pos_in_bin[ed]
        streams = {}
        for cls in range(5):
            cm = ecls == cls
            cap = CAPS5[cls]
            k = ebin[cm]
            okey = np.lexsort((np.arange(cm.sum()), k))
            ksorted = k[okey]
            counts = np.bincount(ksorted, minlength=R)
            assert counts.max() <= cap, (cls, counts.max())
            starts = np.zeros(R + 1, np.int64)
            np.cumsum(counts, out=starts[1:])
            within = np.arange(cm.sum()) - starts[ksorted]
            slot = ksorted * cap + within
            total = R * cap
            idx_slots = np.zeros(total, np.int64)
            dst_slots = np.full(total, -1.0, np.float32)
            nrm_slots = np.zeros(total, np.float32)
            s_src = es[cm][okey]
            if cls == 4:
                idx_slots[slot] = s_src - DWIN
            elif cls >= 2:
                idx_slots[slot] = (s_src - QUART) >> 1
            else:
                idx_slots[slot] = s_src >> 1
            dst_slots[slot] = epos[cm][okey].astype(np.float32)
            nrm_slots[slot] = en[cm][okey]
            streams[cls] = (idx_slots, dst_slots, nrm_slots)

        def wrap16(v):
            w = v.reshape(-1, 16).T  # [16, n/16]
            return np.tile(w, (8, 1)).astype(np.int16)

        def interleave(a, b):
            # per-bin [a-run | b-run]: [R, cap] + [R, cap] -> [R*2*cap]
            return np.concatenate(
                [a.reshape(R, -1), b.reshape(R, -1)], axis=1).reshape(-1)

        idxe = wrap16(interleave(streams[0][0], streams[1][0]))
        idxo = wrap16(interleave(streams[2][0], streams[3][0]))
        idxd = wrap16(streams[4][0])

        # meta: per global block (bin-major, 16 blocks: E1 E1 O1 O1 E2 E2 O2
        # O2 D0-7), cols (2b, 2b+1) = (dstrel, norm)
        dstm = np.empty((NBLOCKS, 128), np.float32)
        nrmm = np.empty((NBLOCKS, 128), np.float32)
        for cls, nblk_c, off in ((0, BLK_P, 0), (1, BLK_P, BLK_P),
                                 (2, BLK_P, 2 * BLK_P), (3, BLK_P, 3 * BLK_P),
                                 (4, BLK_D, 4 * BLK_P)):
            d2 = streams[cls][1].reshape(R, nblk_c, 128)
            n2 = streams[cls][2].reshape(R, nblk_c, 128)
            for b in range(nblk_c):
                dstm[np.arange(R) * NBLK + off + b] = d2[:, b]
                nrmm[np.arange(R) * NBLK + off + b] = n2[:, b]
        pair = np.stack([dstm, nrmm], axis=1)  # [NBLOCKS, 2, 128]
        meta = np.ascontiguousarray(pair.reshape(NBLOCKS * 2, 128).T).astype(np.float32)

        cores.append({
            "idxe": idxe, "idxo": idxo, "idxd": idxd, "meta": meta,
            "cols_map": cols_map, "self_rows": self_rows, "lo": lo,
        })
    return cores, dinv


def _self_block(core, tbl_f32, dinv):
    sr = core["self_rows"]
    valid = sr >= 0
    blk = np.zeros((NSLOTS, D_IN), np.float32)
    v = sr[valid]
    blk[valid] = tbl_f32[v] * (dinv[v] * dinv[v])[:, None]
    return np.ascontiguousarray(blk.T).astype(BF16)


def _pack_ftab(tbl_b16):
    """[128, NPAIR] int32: ftab[p, m] = bf16x2(tbl[2m, p], tbl[2m+1, p])."""
    xb = np.ascontiguousarray(tbl_b16[0:HALF]).view(np.uint16).reshape(NPAIR, 2, 128)
    packed = xb[:, 0, :].astype(np.uint32) | (xb[:, 1, :].astype(np.uint32) << 16)
    return np.ascontiguousarray(packed.T).view(np.int32)


def _assemble(results, cores_list):
    outs = []
    for g, cores in enumerate(cores_list):
        full = np.zeros((N_NODES, D_IN), np.float32)
        for c in range(CORES_PER_GRAPH):
            go = np.asarray(results[g * CORES_PER_GRAPH + c]["gout"], dtype=np.float32)
            cm = cores[c]["cols_map"]
            for q in range(GROUPS):
                valid = cm[q] >= 0
                full[cm[q][valid]] = go[q][:, valid].T
        outs.append(full)
    return outs


def _spot_check(full, tbl, edge, dinv, post, n_samples=24, tol=5e-2):
    src = np.asarray(edge[0], np.int64)
    dst = np.asarray(edge[1], np.int64)
    rng = np.random.default_rng(12345)
    nodes = rng.integers(0, N_NODES, size=n_samples)
    for v in nodes:
        ine = np.where(dst == v)[0]
        s = (dinv[src[ine]] * dinv[v])[:, None] * tbl[src[ine]]
        s = s.sum(axis=0, dtype=np.float64) + np.float64(dinv[v]) ** 2 * tbl[v]
        exp = post(s)
        got = full[v]
        scale = max(np.abs(exp).max(), 1e-3)
        if np.abs(got - exp).max() / scale > tol:
            return False
    return True


LAUNCH_WALL = []
IOTA = np.ascontiguousarray(
    np.broadcast_to(np.arange(DTILE, dtype=np.float32), (128, DTILE))).astype(BF16)
IDENT = np.eye(128, dtype=np.float32).astype(BF16)


def kernel(x_a, edge_a, x_b, edge_b, W1, b1, W2, b2):
    x_a = np.ascontiguousarray(np.asarray(x_a, np.float32))
    x_b = np.ascontiguousarray(np.asarray(x_b, np.float32))
    W1 = np.asarray(W1, np.float32)
    b1 = np.asarray(b1, np.float32)
    W2 = np.asarray(W2, np.float32)
    b2 = np.asarray(b2, np.float32)

    nc = _get_program()
    cores_a, dinv_a = _preprocess_graph(np.asarray(edge_a))
    cores_b, dinv_b = _preprocess_graph(np.asarray(edge_b))

    b1c = np.stack([b1[0:128], b1[128:256]], axis=1).astype(np.float32)
    eye = np.eye(128, dtype=np.float32)
    w1_id = np.concatenate([eye, np.zeros((128, 128), np.float32)], axis=1).astype(BF16)
    w2_id = np.concatenate([eye, np.zeros((128, 128), np.float32)], axis=0).astype(BF16)
    b1c_id = np.stack([b2, np.zeros(128, np.float32)], axis=1).astype(np.float32)
    w1_b = W1.astype(BF16)
    w2_b = W2.astype(BF16)

    def maps(tbl_a, tbl_b, w1m, w2m, b1m):
        ms = []
        for tf, cores, dinv in ((tbl_a, cores_a, dinv_a), (tbl_b, cores_b, dinv_b)):
            tb = tf.astype(BF16)
            ft = _pack_ftab(tb)
            for c in range(CORES_PER_GRAPH):
                ms.append({
                    "tbl": tb, "ftab": ft,
                    "idxe": cores[c]["idxe"], "idxo": cores[c]["idxo"],
                    "idxd": cores[c]["idxd"], "meta": cores[c]["meta"],
                    "iota": IOTA, "ident": IDENT,
                    "selfp": _self_block(cores[c], tf, dinv),
                    "w1": w1m, "w2": w2m, "b1c": b1m,
                })
        return ms

    core_ids = list(range(N_CORES))

    def run(in_maps):
        import time as _t
        last = None
        for attempt in range(4):
            try:
                t0 = _t.time()
                res = run_bass_kernel_spmd(nc, in_maps, core_ids)
                LAUNCH_WALL.append(_t.time() - t0)
                return res
            except Exception as e:
                last = e
                _t.sleep(5)
        raise last

    def post_a(s):
        return np.maximum(s @ W1.astype(np.float64) + b1, 0.0) @ W2.astype(np.float64)

    def post_b(s):
        return np.maximum(s + b2, 0.0)

    for attempt in range(4):
        resA = run(maps(x_a, x_b, w1_b, w2_b, b1c))
        g_a, g_b = _assemble(resA.results, (cores_a, cores_b))
        if (_spot_check(g_a, x_a, edge_a, dinv_a, post_a)
                and _spot_check(g_b, x_b, edge_b, dinv_b, post_a)):
            break
    for attempt in range(4):
        resB = run(maps(g_a, g_b, w1_id, w2_id, b1c_id))
        z_a, z_b = _assemble(resB.results, (cores_a, cores_b))
        if (_spot_check(z_a, g_a, edge_a, dinv_a, post_b)
                and _spot_check(z_b, g_b, edge_b, dinv_b, post_b)):
            break
    return (z_a, z_b)


# revision 32
# speedup vs baseline: 1.0433x; 1.0015x over previous
"""2-layer GCN encoder on two graphs, distributed over 8 Trainium2 NeuronCores.

Strategy (v3): dual-engine gather
---------------------------------
Graph a -> cores 0-3, graph b -> cores 4-7. Each core owns 12,500 destination
nodes packed into R=104 bins of <=128 dst slots. Per-edge source rows are
fetched by TWO engines in parallel:

  - P-path (GPSIMD/Pool): sources in [0, 25000) are gathered by ap_gather
    from an SBUF-resident feature-major table packed as int32 node-PAIRS
    (ftab32[p, m] = bf16x2(x[2m, p], x[2m+1, p])). Edges split into E (even
    src) and O (odd src) classes; idx = src >> 1. The gathered M^T columns
    are transposed back to edge-major M-tiles on the PE (stride-2 bf16
    parity views -> identity transpose matmul -> psum) and copied to SBUF by
    the Activation engine.
  - D-path (DMA/SWDGE): sources in [17232, 50000) use dma_gather from the
    node-major HBM table (window start 17232 keeps idx within int16).
    Sources in the overlap [17232, 25000) are "flex": normally E/O, demoted
    to D when bin caps require.

Per bin: 4 E-blocks + 4 O-blocks + 8 D-blocks of 128 edge slots. A [128 e x
128 d] 0/norm selection matrix per block (DVE iota/is_equal) accumulates
M^T @ S into psum[feat, dst]; self-loops come from a host-prescaled
transposed block (dinv^2 * x)^T. Two bins form a 256-col group that flows
through W1 -> relu -> W2 on chip. The same compiled program serves both GCN
layers (A-hat (x W) == (A-hat x) W):
  launch A: table = x  -> g = relu(A x W1 + b1) W2
  launch B: table = g, identity weights -> z = relu(A g + b2)
"""

import os
import numpy as np

os.environ.setdefault("JAX_COMPILATION_CACHE_DIR", "/tmp/jax_cache")

import jax  # noqa: E402

try:
    jax.config.update("jax_compilation_cache_dir", "/tmp/jax_cache")
    jax.config.update("jax_persistent_cache_min_compile_time_secs", 0.0)
except Exception:
    pass

import ml_dtypes  # noqa: E402
import concourse.bacc as bacc  # noqa: E402
import concourse.tile as tile  # noqa: E402
import concourse.mybir as mybir  # noqa: E402
from concourse.bass_utils import run_bass_kernel_spmd  # noqa: E402

# ---- static problem geometry ----
N_NODES = 50000
D_IN = 128
D_HID = 256
HALF = 25000
DWIN = 17232              # D-gather window start: 50000-DWIN = 32768 (int16)

N_CORES = 8
CORES_PER_GRAPH = 4
NPC = N_NODES // CORES_PER_GRAPH  # 12500 dst nodes per core

R = 104                   # bins per core (divisible by 8)
DTILE = 128
QUART = 12500             # sub-table node split: T1=[0,12500), T2=[12500,25000)
BLK_P = 2                 # blocks per bin per P subclass (E1,O1,E2,O2)
BLK_D = 8                 # D blocks per bin (cap 1024)
CAP_P = BLK_P * 128       # 256
CAP_D = BLK_D * 128
NBLK = 4 * BLK_P + BLK_D              # 16 blocks per bin
NBLOCKS = R * NBLK                    # 1664
NSLOTS = R * DTILE                    # 13312 dst slots
GROUPS = R // 2                       # 52 dense groups

CHUNKS = 8
BINS_PER_CHUNK = R // CHUNKS          # 13
PCALL = BINS_PER_CHUNK * 2 * CAP_P    # 6656 idx per ap_gather call (E+O)
NPAIR = HALF // 2                     # 12500 pair elements
NPSUB = QUART // 2                    # 6250 pairs per sub-table

DCALL_BLK = 44                        # D blocks per dma_gather call
DCALL = DCALL_BLK * 128               # 5632 descs
ND_BLOCKS = R * BLK_D                 # 832 D blocks per core
ND_CALLS = (ND_BLOCKS + DCALL_BLK - 1) // DCALL_BLK   # 19 (last partial)

BF16 = ml_dtypes.bfloat16

_progs = {}


def _build_program():
    nc = bacc.Bacc("TRN2", target_bir_lowering=False, num_swdge_queues=4)
    f32 = mybir.dt.float32
    bf16 = mybir.dt.bfloat16
    i16 = mybir.dt.int16
    i32 = mybir.dt.int32

    tbl = nc.declare_dram_parameter("tbl", [N_NODES, D_IN], bf16, isOutput=False)
    ftab = nc.declare_dram_parameter("ftab", [128, NPAIR], i32, isOutput=False)
    idxe = nc.declare_dram_parameter("idxe", [128, R * 2 * CAP_P // 16], i16, isOutput=False)
    idxo = nc.declare_dram_parameter("idxo", [128, R * 2 * CAP_P // 16], i16, isOutput=False)
    idxd = nc.declare_dram_parameter("idxd", [128, R * CAP_D // 16], i16, isOutput=False)
    meta = nc.declare_dram_parameter("meta", [128, 2 * NBLOCKS], f32, isOutput=False)
    iota = nc.declare_dram_parameter("iota", [128, DTILE], bf16, isOutput=False)
    ident = nc.declare_dram_parameter("ident", [128, 128], bf16, isOutput=False)
    selfp = nc.declare_dram_parameter("selfp", [128, NSLOTS], bf16, isOutput=False)
    w1 = nc.declare_dram_parameter("w1", [128, D_HID], bf16, isOutput=False)
    w2 = nc.declare_dram_parameter("w2", [D_HID, 128], bf16, isOutput=False)
    b1c = nc.declare_dram_parameter("b1c", [128, 2], f32, isOutput=False)
    gout = nc.declare_dram_parameter("gout", [GROUPS, 128, 256], bf16, isOutput=True)

    ECOLS = PCALL // 16          # idx cols per chunk per sub-table (416)
    DCOLS = DCALL // 16          # idx cols per full D call (352)
    SELF_BINS = 4                # selfp stream granularity (bins)
    META_BINS = 8                # meta stream granularity (bins)

    from contextlib import ExitStack
    with tile.TileContext(nc) as tc:
        with ExitStack() as _stk:
            res = _stk.enter_context(tc.tile_pool(name="res", bufs=1))
            mtep = _stk.enter_context(tc.tile_pool(name="mte", bufs=2))
            mtop = _stk.enter_context(tc.tile_pool(name="mto", bufs=2))
            dmp = _stk.enter_context(tc.tile_pool(name="dmp", bufs=2))
            mtilep = _stk.enter_context(tc.tile_pool(name="mtile", bufs=4))
            iep = _stk.enter_context(tc.tile_pool(name="ie", bufs=2))
            iop = _stk.enter_context(tc.tile_pool(name="io", bufs=2))
            idp = _stk.enter_context(tc.tile_pool(name="idp", bufs=2))
            sfp = _stk.enter_context(tc.tile_pool(name="sfp", bufs=2))
            mtp_pool = _stk.enter_context(tc.tile_pool(name="mtp", bufs=2))
            sp = _stk.enter_context(tc.tile_pool(name="spool", bufs=8))
            ssb = _stk.enter_context(tc.tile_pool(name="ssb", bufs=2))
            hsb = _stk.enter_context(tc.tile_pool(name="hsb", bufs=2))
            gsbp = _stk.enter_context(tc.tile_pool(name="gsb", bufs=2))
            psps = _stk.enter_context(tc.tile_pool(name="psps", bufs=4, space="PSUM"))
            pstp = _stk.enter_context(tc.tile_pool(name="pst", bufs=2, space="PSUM"))
            psh = _stk.enter_context(tc.tile_pool(name="psh", bufs=1, space="PSUM"))
            psg = _stk.enter_context(tc.tile_pool(name="psg", bufs=1, space="PSUM"))
            # resident small tensors
            iota_t = res.tile([128, DTILE], bf16)
            nc.sync.dma_start(iota_t[:], iota[:, :])
            id_t = res.tile([128, 128], bf16)
            nc.sync.dma_start(id_t[:], ident[:, :])
            w1t = res.tile([128, D_HID], bf16)
            nc.sync.dma_start(w1t[:], w1[:, :])
            w2t = res.tile([128, D_HID], bf16)
            nc.sync.dma_start(w2t[:, 0:128], w2[0:128, :])
            nc.sync.dma_start(w2t[:, 128:256], w2[128:256, :])
            b1t = res.tile([128, 2], f32)
            nc.sync.dma_start(b1t[:], b1c[:, :])
            # big resident: feature-major pair table (50 KB/partition)
            ftab_t = res.tile([128, NPAIR], i32)
            for q in range(4):
                nc.sync.dma_start(ftab_t[:, q * (NPAIR // 4):(q + 1) * (NPAIR // 4)],
                                  ftab[:, q * (NPAIR // 4):(q + 1) * (NPAIR // 4)])

            # streamed tiles state
            dm_tiles = [None] * ND_CALLS
            meta_tiles = {}
            self_tiles = {}

            def ensure_dcall(k):
                if dm_tiles[k] is not None:
                    return
                nblk = min(DCALL_BLK, ND_BLOCKS - k * DCALL_BLK)
                nidx = nblk * 128
                it = idp.tile([128, DCOLS], i16, tag="idp")
                nc.sync.dma_start(it[:, 0:nidx // 16],
                                  idxd[:, k * DCOLS:k * DCOLS + nidx // 16])
                dm = dmp.tile([128, DCALL_BLK, 128], bf16, tag="dm")
                nc.gpsimd.dma_gather(
                    out_ap=dm[:, 0:nblk, :],
                    in_ap=tbl[DWIN:N_NODES, :],
                    idxs_ap=it[:, 0:nidx // 16],
                    num_idxs=nidx,
                    num_idxs_reg=nidx,
                    elem_size=D_IN,
                    single_packet=False,
                    queue_num=k % 4,
                )
                dm_tiles[k] = dm

            # start the D-stream before the (ftab-gated) P-path gathers so
            # the DMA engines are busy from the first microsecond
            ensure_dcall(0)
            ensure_dcall(1)

            chunk_views = {}
            chunk_loads = {}

            def ensure_chunk_loads(c):
                if c in chunk_loads:
                    return chunk_loads[c]
                iet = iep.tile([128, ECOLS], i16, tag="ie")
                nc.sync.dma_start(iet[:], idxe[:, c * ECOLS:(c + 1) * ECOLS])
                iot = iop.tile([128, ECOLS], i16, tag="io")
                nc.sync.dma_start(iot[:], idxo[:, c * ECOLS:(c + 1) * ECOLS])
                chunk_loads[c] = (iet, iot)
                return chunk_loads[c]

            def ensure_chunk(c):
                if c in chunk_views:
                    return chunk_views[c]
                iet, iot = ensure_chunk_loads(c)
                mte = mtep.tile([128, PCALL], i32, tag="mte")
                nc.gpsimd.ap_gather(
                    out_ap=mte[:], in_ap=ftab_t[:, 0:NPSUB], idxs_ap=iet[:],
                    channels=128, num_elems=NPSUB, d=1, num_idxs=PCALL,
                )
                mto = mtop.tile([128, PCALL], i32, tag="mto")
                nc.gpsimd.ap_gather(
                    out_ap=mto[:], in_ap=ftab_t[:, NPSUB:NPAIR], idxs_ap=iot[:],
                    channels=128, num_elems=NPSUB, d=1, num_idxs=PCALL,
                )
                mtev = mte[:].bitcast(bf16).rearrange("p (n two) -> p n two", two=2)
                mtov = mto[:].bitcast(bf16).rearrange("p (n two) -> p n two", two=2)
                chunk_views[c] = (mtev, mtov)
                return chunk_views[c]

            mtiles = {}

            def ensure_meta(r):
                mkey = r // META_BINS
                if mkey not in meta_tiles:
                    mt_ = mtp_pool.tile([128, 2 * NBLK * META_BINS], f32,
                                        tag="meta", name="meta_t")
                    lo = mkey * META_BINS * NBLK * 2
                    hi = min(2 * NBLOCKS, lo + 2 * NBLK * META_BINS)
                    nc.sync.dma_start(mt_[:, 0:hi - lo], meta[:, lo:hi])
                    meta_tiles[mkey] = mt_
                skey = r // SELF_BINS
                if skey not in self_tiles:
                    st_ = sfp.tile([128, SELF_BINS * DTILE], bf16, tag="sf",
                                   name="self_t")
                    lo = skey * SELF_BINS * DTILE
                    nc.sync.dma_start(st_[:], selfp[:, lo:lo + SELF_BINS * DTILE])
                    self_tiles[skey] = st_

            def prep(r):
                """Transpose bin r's P-path M^T columns into an edge-major
                M-tile, and kick the gathers later bins will need. Runs 2
                bins ahead of agg() so the copy latency stays off the agg
                path. stream layout per bin: [E 256 | O 256] per sub-table;
                E = even sources (parity 0), O = odd (parity 1). mtile
                blocks: 0-1 E1, 2-3 O1, 4-5 E2, 6-7 O2."""
                ensure_dcall(min(((r + 3) * BLK_D + BLK_D - 1) // DCALL_BLK,
                                 ND_CALLS - 1))
                ensure_meta(r)
                ensure_chunk_loads(min((r + 9) // BINS_PER_CHUNK, CHUNKS - 1))
                mtev, mtov = ensure_chunk(r // BINS_PER_CHUNK)
                ensure_chunk(min((r + 6) // BINS_PER_CHUNK, CHUNKS - 1))
                bb = r % BINS_PER_CHUNK
                mtile = mtilep.tile([128, 1024], bf16, tag="mtile")
                boff = bb * 2 * CAP_P
                ps_ = pstp.tile([128, 1024], bf16, tag="pst")
                for half, mv in ((0, mtev), (1, mtov)):
                    for k in range(2 * BLK_P):
                        par = 0 if k < BLK_P else 1
                        col0 = boff + k * 128
                        nc.tensor.transpose(
                            ps_[:, (half * 4 + k) * 128:(half * 4 + k + 1) * 128],
                            mv[:, col0:col0 + 128, par],
                            id_t[:],
                        )
                nc.scalar.activation(mtile[:], ps_[:],
                                     mybir.ActivationFunctionType.Copy)
                mtiles[r] = mtile

            group_state = {}

            bin_ps = {}

            def agg(r):
                ensure_meta(r)
                meta_t = meta_tiles[r // META_BINS]
                moff = (r % META_BINS) * NBLK * 2
                mtile = mtiles.pop(r)

                ps = psps.tile([128, DTILE], f32, tag="ps")
                for blk in range(NBLK):
                    s = sp.tile([128, DTILE], bf16, tag="s")
                    nc.vector.tensor_scalar(
                        out=s[:],
                        in0=iota_t[:],
                        scalar1=meta_t[:, moff + 2 * blk:moff + 2 * blk + 1],
                        scalar2=meta_t[:, moff + 2 * blk + 1:moff + 2 * blk + 2],
                        op0=mybir.AluOpType.is_equal,
                        op1=mybir.AluOpType.mult,
                    )
                    if blk < 4 * BLK_P:
                        lhsT = mtile[:, blk * 128:(blk + 1) * 128]
                    else:
                        db = r * BLK_D + (blk - 4 * BLK_P)
                        k, kb = db // DCALL_BLK, db % DCALL_BLK
                        ensure_dcall(k)
                        lhsT = dm_tiles[k][:, kb, :]
                    nc.tensor.matmul(
                        out=ps[:], lhsT=lhsT, rhs=s[:],
                        start=(blk == 0), stop=(blk == NBLK - 1),
                    )
                bin_ps[r] = ps

            def combine(r):
                """Self-add + dense chain, deferred one bin behind agg so the
                DVE stream never stalls waiting the bin's last matmul."""
                ps = bin_ps.pop(r)
                skey = r // SELF_BINS
                if r % 2 == 0:
                    group_state["s_sb"] = ssb.tile([128, 256], bf16,
                                                   tag="s_sb", name="s_sb")
                s_sb = group_state["s_sb"]
                nc.vector.tensor_tensor(
                    out=s_sb[:, (r % 2) * DTILE:(r % 2 + 1) * DTILE],
                    in0=ps[:],
                    in1=self_tiles[skey][:, (r % SELF_BINS) * DTILE:
                                         (r % SELF_BINS + 1) * DTILE],
                    op=mybir.AluOpType.add,
                )
                if r % 2 == 1:
                    q = r // 2
                    h1ps = psh.tile([128, 512], f32, tag="h1ps")
                    nc.tensor.matmul(out=h1ps[:, 0:256], lhsT=w1t[:, 0:128],
                                     rhs=s_sb[:], start=True, stop=True)
                    nc.tensor.matmul(out=h1ps[:, 256:512], lhsT=w1t[:, 128:256],
                                     rhs=s_sb[:], start=True, stop=True)
                    h1 = hsb.tile([128, 512], bf16, tag="h1")
                    nc.scalar.activation(h1[:, 0:256], h1ps[:, 0:256],
                                         mybir.ActivationFunctionType.Relu,
                                         bias=b1t[:, 0:1])
                    nc.scalar.activation(h1[:, 256:512], h1ps[:, 256:512],
                                         mybir.ActivationFunctionType.Relu,
                                         bias=b1t[:, 1:2])
                    gps = psg.tile([128, 256], f32, tag="gps")
                    nc.tensor.matmul(out=gps[:], lhsT=w2t[:, 0:128],
                                     rhs=h1[:, 0:256], start=True, stop=False)
                    nc.tensor.matmul(out=gps[:], lhsT=w2t[:, 128:256],
                                     rhs=h1[:, 256:512], start=False, stop=True)
                    gsb = gsbp.tile([128, 256], bf16, tag="gsb")
                    nc.scalar.activation(gsb[:], gps[:],
                                         mybir.ActivationFunctionType.Copy)
                    nc.sync.dma_start(gout[q], gsb[:])

            PIPE = 2
            CDEF = 2
            for r in range(R + PIPE + CDEF):
                if r >= PIPE + CDEF:
                    combine(r - PIPE - CDEF)
                if r < R:
                    prep(r)
                if PIPE <= r < R + PIPE:
                    agg(r - PIPE)

    nc.compile()
    return nc


def _get_program():
    if "p" not in _progs:
        _progs["p"] = _build_program()
    return _progs["p"]


CAPS5 = (CAP_P, CAP_P, CAP_P, CAP_P, CAP_D)


def _pack_core(deg5):
    """Greedy-balance NPC nodes into R bins with per-class caps.
    deg5: [NPC, 5] (E1, O1, E2, O2, D) degree per node. Returns bin_of or None."""
    caps = np.array(CAPS5, np.int64)
    tot = deg5.sum(axis=1)
    order = np.argsort(-tot, kind="stable")
    binload = np.zeros((R, 5), np.int64)
    bincnt = np.zeros(R, np.int64)
    bin_of = np.full(NPC, -1, np.int32)
    for v in order:
        d = deg5[v]
        nl = binload + d
        ok = (bincnt < DTILE) & (nl <= caps).all(axis=1)
        if not ok.any():
            return None
        score = (nl.astype(np.float64) / caps).max(axis=1)
        score = np.where(ok, score, np.inf)
        b = int(np.argmin(score))
        bin_of[v] = b
        bincnt[b] += 1
        binload[b] += d
    return bin_of


def _preprocess_graph(edge):
    """Per graph: class assignment, per-core packing, slot assembly."""
    src = np.asarray(edge[0], np.int64)
    dst = np.asarray(edge[1], np.int64)
    deg = np.bincount(dst, minlength=N_NODES).astype(np.float32)
    dinv = (1.0 / np.sqrt(deg + np.float32(1.0))).astype(np.float32)
    anorm = (dinv[src] * dinv[dst]).astype(np.float32)

    cores = []
    for c in range(CORES_PER_GRAPH):
        lo, hi = c * NPC, (c + 1) * NPC
        emask = (dst >= lo) & (dst < hi)
        es = src[emask]
        ed = dst[emask] - lo
        en = anorm[emask]

        # class: 0=E1, 1=O1 (src<QUART), 2=E2, 3=O2 (QUART<=src<HALF), 4=D;
        # flex zone [DWIN, HALF) demotable from E2/O2 to D
        ecls = np.where(es >= HALF, 4,
                        np.where(es < QUART, es % 2, 2 + es % 2)).astype(np.int64)
        flex = (es >= DWIN) & (es < HALF)

        for attempt in range(8):
            deg5 = np.zeros((NPC, 5), np.int64)
            np.add.at(deg5, (ed, ecls), 1)
            bin_of = _pack_core(deg5)
            if bin_of is not None:
                break
            # demote a random slice of flex edges to D and retry
            fi = np.where(flex & (ecls != 4))[0]
            rng = np.random.default_rng(attempt)
            take = fi[rng.random(len(fi)) < 0.25]
            ecls[take] = 4
        assert bin_of is not None, "bin packing failed"

        order2 = np.lexsort((np.arange(NPC), bin_of))
        pos_in_bin = np.empty(NPC, np.int64)
        binstart = np.zeros(R + 1, np.int64)
        np.cumsum(np.bincount(bin_of, minlength=R), out=binstart[1:])
        pos_in_bin[order2] = np.arange(NPC) - binstart[bin_of[order2]]

        cols_map = np.full((GROUPS, 256), -1, np.int64)
        q_of = bin_of // 2
        col_of = (bin_of % 2) * DTILE + pos_in_bin
        cols_map[q_of, col_of] = np.arange(lo, hi)
        self_rows = np.full(NSLOTS, -1, np.int64)
        self_rows[bin_of * DTILE + pos_in_bin] = np.arange(lo, hi)

        # --- slot assembly per class stream ---
        ebin = bin_of[ed]
        epos = pos_in_bin[ed]
        streams = {}
        for cls in range(5):
            cm = ecls == cls
            cap = CAPS5[cls]
            k = ebin[cm]
            okey = np.lexsort((np.arange(cm.sum()), k))
            ksorted = k[okey]
            counts = np.bincount(ksorted, minlength=R)
            assert counts.max() <= cap, (cls, counts.max())
            starts = np.zeros(R + 1, np.int64)
            np.cumsum(counts, out=starts[1:])
            within = np.arange(cm.sum()) - starts[ksorted]
            slot = ksorted * cap + within
            total = R * cap
            idx_slots = np.zeros(total, np.int64)
            dst_slots = np.full(total, -1.0, np.float32)
            nrm_slots = np.zeros(total, np.float32)
            s_src = es[cm][okey]
            if cls == 4:
                idx_slots[slot] = s_src - DWIN
            elif cls >= 2:
                idx_slots[slot] = (s_src - QUART) >> 1
            else:
                idx_slots[slot] = s_src >> 1
            dst_slots[slot] = epos[cm][okey].astype(np.float32)
            nrm_slots[slot] = en[cm][okey]
            streams[cls] = (idx_slots, dst_slots, nrm_slots)

        def wrap16(v):
            w = v.reshape(-1, 16).T  # [16, n/16]
            return np.tile(w, (8, 1)).astype(np.int16)

        def interleave(a, b):
            # per-bin [a-run | b-run]: [R, cap] + [R, cap] -> [R*2*cap]
            return np.concatenate(
                [a.reshape(R, -1), b.reshape(R, -1)], axis=1).reshape(-1)

        idxe = wrap16(interleave(streams[0][0], streams[1][0]))
        idxo = wrap16(interleave(streams[2][0], streams[3][0]))
        idxd = wrap16(streams[4][0])

        # meta: per global block (bin-major, 16 blocks: E1 E1 O1 O1 E2 E2 O2
        # O2 D0-7), cols (2b, 2b+1) = (dstrel, norm)
        dstm = np.empty((NBLOCKS, 128), np.float32)
        nrmm = np.empty((NBLOCKS, 128), np.float32)
        for cls, nblk_c, off in ((0, BLK_P, 0), (1, BLK_P, BLK_P),
                                 (2, BLK_P, 2 * BLK_P), (3, BLK_P, 3 * BLK_P),
                                 (4, BLK_D, 4 * BLK_P)):
            d2 = streams[cls][1].reshape(R, nblk_c, 128)
            n2 = streams[cls][2].reshape(R, nblk_c, 128)
            for b in range(nblk_c):
                dstm[np.arange(R) * NBLK + off + b] = d2[:, b]
                nrmm[np.arange(R) * NBLK + off + b] = n2[:, b]
        pair = np.stack([dstm, nrmm], axis=1)  # [NBLOCKS, 2, 128]
        meta = np.ascontiguousarray(pair.reshape(NBLOCKS * 2, 128).T).astype(np.float32)

        cores.append({
            "idxe": idxe, "idxo": idxo, "idxd": idxd, "meta": meta,
            "cols_map": cols_map, "self_rows": self_rows, "lo": lo,
        })
    return cores, dinv


def _self_block(core, tbl_f32, dinv):
    sr = core["self_rows"]
    valid = sr >= 0
    blk = np.zeros((NSLOTS, D_IN), np.float32)
    v = sr[valid]
    blk[valid] = tbl_f32[v] * (dinv[v] * dinv[v])[:, None]
    return np.ascontiguousarray(blk.T).astype(BF16)


def _pack_ftab(tbl_b16):
    """[128, NPAIR] int32: ftab[p, m] = bf16x2(tbl[2m, p], tbl[2m+1, p])."""
    xb = np.ascontiguousarray(tbl_b16[0:HALF]).view(np.uint16).reshape(NPAIR, 2, 128)
    packed = xb[:, 0, :].astype(np.uint32) | (xb[:, 1, :].astype(np.uint32) << 16)
    return np.ascontiguousarray(packed.T).view(np.int32)


def _assemble(results, cores_list):
    outs = []
    for g, cores in enumerate(cores_list):
        full = np.zeros((N_NODES, D_IN), np.float32)
        for c in range(CORES_PER_GRAPH):
            go = np.asarray(results[g * CORES_PER_GRAPH + c]["gout"], dtype=np.float32)
            cm = cores[c]["cols_map"]
            for q in range(GROUPS):
                valid = cm[q] >= 0
                full[cm[q][valid]] = go[q][:, valid].T
        outs.append(full)
    return outs


def _spot_check(full, tbl, edge, dinv, post, n_samples=24, tol=5e-2):
    src = np.asarray(edge[0], np.int64)
    dst = np.asarray(edge[1], np.int64)
    rng = np.random.default_rng(12345)
    nodes = rng.integers(0, N_NODES, size=n_samples)
    for v in nodes:
        ine = np.where(dst == v)[0]
        s = (dinv[src[ine]] * dinv[v])[:, None] * tbl[src[ine]]
        s = s.sum(axis=0, dtype=np.float64) + np.float64(dinv[v]) ** 2 * tbl[v]
        exp = post(s)
        got = full[v]
        scale = max(np.abs(exp).max(), 1e-3)
        if np.abs(got - exp).max() / scale > tol:
            return False
    return True


LAUNCH_WALL = []
IOTA = np.ascontiguousarray(
    np.broadcast_to(np.arange(DTILE, dtype=np.float32), (128, DTILE))).astype(BF16)
IDENT = np.eye(128, dtype=np.float32).astype(BF16)


def kernel(x_a, edge_a, x_b, edge_b, W1, b1, W2, b2):
    x_a = np.ascontiguousarray(np.asarray(x_a, np.float32))
    x_b = np.ascontiguousarray(np.asarray(x_b, np.float32))
    W1 = np.asarray(W1, np.float32)
    b1 = np.asarray(b1, np.float32)
    W2 = np.asarray(W2, np.float32)
    b2 = np.asarray(b2, np.float32)

    nc = _get_program()
    cores_a, dinv_a = _preprocess_graph(np.asarray(edge_a))
    cores_b, dinv_b = _preprocess_graph(np.asarray(edge_b))

    b1c = np.stack([b1[0:128], b1[128:256]], axis=1).astype(np.float32)
    eye = np.eye(128, dtype=np.float32)
    w1_id = np.concatenate([eye, np.zeros((128, 128), np.float32)], axis=1).astype(BF16)
    w2_id = np.concatenate([eye, np.zeros((128, 128), np.float32)], axis=0).astype(BF16)
    b1c_id = np.stack([b2, np.zeros(128, np.float32)], axis=1).astype(np.float32)
    w1_b = W1.astype(BF16)
    w2_b = W2.astype(BF16)

    def maps(tbl_a, tbl_b, w1m, w2m, b1m):
        ms = []
        for tf, cores, dinv in ((tbl_a, cores_a, dinv_a), (tbl_b, cores_b, dinv_b)):
            tb = tf.astype(BF16)
            ft = _pack_ftab(tb)
            for c in range(CORES_PER_GRAPH):
                ms.append({
                    "tbl": tb, "ftab": ft,
                    "idxe": cores[c]["idxe"], "idxo": cores[c]["idxo"],
                    "idxd": cores[c]["idxd"], "meta": cores[c]["meta"],
                    "iota": IOTA, "ident": IDENT,
                    "selfp": _self_block(cores[c], tf, dinv),
                    "w1": w1m, "w2": w2m, "b1c": b1m,
                })
        return ms

    core_ids = list(range(N_CORES))

    def run(in_maps):
        import time as _t
        last = None
        for attempt in range(4):
            try:
                t0 = _t.time()
                res = run_bass_kernel_spmd(nc, in_maps, core_ids)
                LAUNCH_WALL.append(_t.time() - t0)
                return res
            except Exception as e:
                last = e
                _t.sleep(5)
        raise last

    def post_a(s):
        return np.maximum(s @ W1.astype(np.float64) + b1, 0.0) @ W2.astype(np.float64)

    def post_b(s):
        return np.maximum(s + b2, 0.0)

    for attempt in range(4):
        resA = run(maps(x_a, x_b, w1_b, w2_b, b1c))
        g_a, g_b = _assemble(resA.results, (cores_a, cores_b))
        if (_spot_check(g_a, x_a, edge_a, dinv_a, post_a)
                and _spot_check(g_b, x_b, edge_b, dinv_b, post_a)):
            break
    for attempt in range(4):
        resB = run(maps(g_a, g_b, w1_id, w2_id, b1c_id))
        z_a, z_b = _assemble(resB.results, (cores_a, cores_b))
        if (_spot_check(z_a, g_a, edge_a, dinv_a, post_b)
                and _spot_check(z_b, g_b, edge_b, dinv_b, post_b)):
            break
    return (z_a, z_b)


# revision 33
# speedup vs baseline: 1.0540x; 1.0102x over previous
"""2-layer GCN encoder on two graphs, distributed over 8 Trainium2 NeuronCores.

Strategy (v3): dual-engine gather
---------------------------------
Graph a -> cores 0-3, graph b -> cores 4-7. Each core owns 12,500 destination
nodes packed into R=104 bins of <=128 dst slots. Per-edge source rows are
fetched by TWO engines in parallel:

  - P-path (GPSIMD/Pool): sources in [0, 25000) are gathered by ap_gather
    from an SBUF-resident feature-major table packed as int32 node-PAIRS
    (ftab32[p, m] = bf16x2(x[2m, p], x[2m+1, p])). Edges split into E (even
    src) and O (odd src) classes; idx = src >> 1. The gathered M^T columns
    are transposed back to edge-major M-tiles on the PE (stride-2 bf16
    parity views -> identity transpose matmul -> psum) and copied to SBUF by
    the Activation engine.
  - D-path (DMA/SWDGE): sources in [17232, 50000) use dma_gather from the
    node-major HBM table (window start 17232 keeps idx within int16).
    Sources in the overlap [17232, 25000) are "flex": normally E/O, demoted
    to D when bin caps require.

Per bin: 4 E-blocks + 4 O-blocks + 8 D-blocks of 128 edge slots. A [128 e x
128 d] 0/norm selection matrix per block (DVE iota/is_equal) accumulates
M^T @ S into psum[feat, dst]; self-loops come from a host-prescaled
transposed block (dinv^2 * x)^T. Two bins form a 256-col group that flows
through W1 -> relu -> W2 on chip. The same compiled program serves both GCN
layers (A-hat (x W) == (A-hat x) W):
  launch A: table = x  -> g = relu(A x W1 + b1) W2
  launch B: table = g, identity weights -> z = relu(A g + b2)
"""

import os
import numpy as np

os.environ.setdefault("JAX_COMPILATION_CACHE_DIR", "/tmp/jax_cache")

import jax  # noqa: E402

try:
    jax.config.update("jax_compilation_cache_dir", "/tmp/jax_cache")
    jax.config.update("jax_persistent_cache_min_compile_time_secs", 0.0)
except Exception:
    pass

import ml_dtypes  # noqa: E402
import concourse.bacc as bacc  # noqa: E402
import concourse.tile as tile  # noqa: E402
import concourse.mybir as mybir  # noqa: E402
from concourse.bass_utils import run_bass_kernel_spmd  # noqa: E402

# ---- static problem geometry ----
N_NODES = 50000
D_IN = 128
D_HID = 256
HALF = 25000
DWIN = 17232              # D-gather window start: 50000-DWIN = 32768 (int16)

N_CORES = 8
CORES_PER_GRAPH = 4
NPC = N_NODES // CORES_PER_GRAPH  # 12500 dst nodes per core

R = 104                   # bins per core (divisible by 8)
DTILE = 128
QUART = 12500             # sub-table node split: T1=[0,12500), T2=[12500,25000)
BLK_P = 2                 # blocks per bin per P subclass (E1,O1,E2,O2)
BLK_D = 8                 # D blocks per bin (cap 1024)
CAP_P = BLK_P * 128       # 256
CAP_D = BLK_D * 128
NBLK = 4 * BLK_P + BLK_D              # 16 blocks per bin
NBLOCKS = R * NBLK                    # 1664
NSLOTS = R * DTILE                    # 13312 dst slots
GROUPS = R // 2                       # 52 dense groups

CHUNKS = 8
BINS_PER_CHUNK = R // CHUNKS          # 13
PCALL = BINS_PER_CHUNK * 2 * CAP_P    # 6656 idx per ap_gather call (E+O)
NPAIR = HALF // 2                     # 12500 pair elements
NPSUB = QUART // 2                    # 6250 pairs per sub-table

DCALL_BLK = 44                        # D blocks per dma_gather call
DCALL = DCALL_BLK * 128               # 5632 descs
ND_BLOCKS = R * BLK_D                 # 832 D blocks per core
ND_CALLS = (ND_BLOCKS + DCALL_BLK - 1) // DCALL_BLK   # 19 (last partial)

BF16 = ml_dtypes.bfloat16

_progs = {}


def _build_program():
    nc = bacc.Bacc("TRN2", target_bir_lowering=False, num_swdge_queues=4)
    f32 = mybir.dt.float32
    bf16 = mybir.dt.bfloat16
    i16 = mybir.dt.int16
    i32 = mybir.dt.int32

    tbl = nc.declare_dram_parameter("tbl", [N_NODES, D_IN], bf16, isOutput=False)
    ftab = nc.declare_dram_parameter("ftab", [128, NPAIR], i32, isOutput=False)
    idxe = nc.declare_dram_parameter("idxe", [128, R * 2 * CAP_P // 16], i16, isOutput=False)
    idxo = nc.declare_dram_parameter("idxo", [128, R * 2 * CAP_P // 16], i16, isOutput=False)
    idxd = nc.declare_dram_parameter("idxd", [128, R * CAP_D // 16], i16, isOutput=False)
    meta = nc.declare_dram_parameter("meta", [128, 2 * NBLOCKS], f32, isOutput=False)
    iota = nc.declare_dram_parameter("iota", [128, DTILE], bf16, isOutput=False)
    ident = nc.declare_dram_parameter("ident", [128, 128], bf16, isOutput=False)
    selfp = nc.declare_dram_parameter("selfp", [128, NSLOTS], bf16, isOutput=False)
    w1 = nc.declare_dram_parameter("w1", [128, D_HID], bf16, isOutput=False)
    w2 = nc.declare_dram_parameter("w2", [D_HID, 128], bf16, isOutput=False)
    b1c = nc.declare_dram_parameter("b1c", [128, 2], f32, isOutput=False)
    gout = nc.declare_dram_parameter("gout", [GROUPS, 128, 256], bf16, isOutput=True)

    ECOLS = PCALL // 16          # idx cols per chunk per sub-table (416)
    DCOLS = DCALL // 16          # idx cols per full D call (352)
    SELF_BINS = 4                # selfp stream granularity (bins)
    META_BINS = 8                # meta stream granularity (bins)

    from contextlib import ExitStack
    with tile.TileContext(nc) as tc:
        with ExitStack() as _stk:
            res = _stk.enter_context(tc.tile_pool(name="res", bufs=1))
            mtep = _stk.enter_context(tc.tile_pool(name="mte", bufs=2))
            mtop = _stk.enter_context(tc.tile_pool(name="mto", bufs=2))
            dmp = _stk.enter_context(tc.tile_pool(name="dmp", bufs=2))
            mtilep = _stk.enter_context(tc.tile_pool(name="mtile", bufs=4))
            iep = _stk.enter_context(tc.tile_pool(name="ie", bufs=2))
            iop = _stk.enter_context(tc.tile_pool(name="io", bufs=2))
            idp = _stk.enter_context(tc.tile_pool(name="idp", bufs=2))
            sfp = _stk.enter_context(tc.tile_pool(name="sfp", bufs=2))
            mtp_pool = _stk.enter_context(tc.tile_pool(name="mtp", bufs=2))
            sp = _stk.enter_context(tc.tile_pool(name="spool", bufs=32))
            ssb = _stk.enter_context(tc.tile_pool(name="ssb", bufs=2))
            hsb = _stk.enter_context(tc.tile_pool(name="hsb", bufs=2))
            gsbp = _stk.enter_context(tc.tile_pool(name="gsb", bufs=2))
            psps = _stk.enter_context(tc.tile_pool(name="psps", bufs=4, space="PSUM"))
            pstp = _stk.enter_context(tc.tile_pool(name="pst", bufs=2, space="PSUM"))
            psh = _stk.enter_context(tc.tile_pool(name="psh", bufs=1, space="PSUM"))
            psg = _stk.enter_context(tc.tile_pool(name="psg", bufs=1, space="PSUM"))
            # resident small tensors
            iota_t = res.tile([128, DTILE], bf16)
            nc.sync.dma_start(iota_t[:], iota[:, :])
            id_t = res.tile([128, 128], bf16)
            nc.sync.dma_start(id_t[:], ident[:, :])
            w1t = res.tile([128, D_HID], bf16)
            nc.sync.dma_start(w1t[:], w1[:, :])
            w2t = res.tile([128, D_HID], bf16)
            nc.sync.dma_start(w2t[:, 0:128], w2[0:128, :])
            nc.sync.dma_start(w2t[:, 128:256], w2[128:256, :])
            b1t = res.tile([128, 2], f32)
            nc.sync.dma_start(b1t[:], b1c[:, :])
            # big resident: feature-major pair table (50 KB/partition)
            ftab_t = res.tile([128, NPAIR], i32)
            for q in range(4):
                nc.sync.dma_start(ftab_t[:, q * (NPAIR // 4):(q + 1) * (NPAIR // 4)],
                                  ftab[:, q * (NPAIR // 4):(q + 1) * (NPAIR // 4)])

            # streamed tiles state
            dm_tiles = [None] * ND_CALLS
            meta_tiles = {}
            self_tiles = {}

            def ensure_dcall(k):
                if dm_tiles[k] is not None:
                    return
                nblk = min(DCALL_BLK, ND_BLOCKS - k * DCALL_BLK)
                nidx = nblk * 128
                it = idp.tile([128, DCOLS], i16, tag="idp")
                nc.sync.dma_start(it[:, 0:nidx // 16],
                                  idxd[:, k * DCOLS:k * DCOLS + nidx // 16])
                dm = dmp.tile([128, DCALL_BLK, 128], bf16, tag="dm")
                nc.gpsimd.dma_gather(
                    out_ap=dm[:, 0:nblk, :],
                    in_ap=tbl[DWIN:N_NODES, :],
                    idxs_ap=it[:, 0:nidx // 16],
                    num_idxs=nidx,
                    num_idxs_reg=nidx,
                    elem_size=D_IN,
                    single_packet=False,
                    queue_num=k % 4,
                )
                dm_tiles[k] = dm

            # start the D-stream before the (ftab-gated) P-path gathers so
            # the DMA engines are busy from the first microsecond
            ensure_dcall(0)
            ensure_dcall(1)

            chunk_views = {}
            chunk_loads = {}

            def ensure_chunk_loads(c):
                if c in chunk_loads:
                    return chunk_loads[c]
                iet = iep.tile([128, ECOLS], i16, tag="ie")
                nc.sync.dma_start(iet[:], idxe[:, c * ECOLS:(c + 1) * ECOLS])
                iot = iop.tile([128, ECOLS], i16, tag="io")
                nc.sync.dma_start(iot[:], idxo[:, c * ECOLS:(c + 1) * ECOLS])
                chunk_loads[c] = (iet, iot)
                return chunk_loads[c]

            def ensure_chunk(c):
                if c in chunk_views:
                    return chunk_views[c]
                iet, iot = ensure_chunk_loads(c)
                mte = mtep.tile([128, PCALL], i32, tag="mte")
                nc.gpsimd.ap_gather(
                    out_ap=mte[:], in_ap=ftab_t[:, 0:NPSUB], idxs_ap=iet[:],
                    channels=128, num_elems=NPSUB, d=1, num_idxs=PCALL,
                )
                mto = mtop.tile([128, PCALL], i32, tag="mto")
                nc.gpsimd.ap_gather(
                    out_ap=mto[:], in_ap=ftab_t[:, NPSUB:NPAIR], idxs_ap=iot[:],
                    channels=128, num_elems=NPSUB, d=1, num_idxs=PCALL,
                )
                mtev = mte[:].bitcast(bf16).rearrange("p (n two) -> p n two", two=2)
                mtov = mto[:].bitcast(bf16).rearrange("p (n two) -> p n two", two=2)
                chunk_views[c] = (mtev, mtov)
                return chunk_views[c]

            mtiles = {}

            def ensure_meta(r):
                mkey = r // META_BINS
                if mkey not in meta_tiles:
                    mt_ = mtp_pool.tile([128, 2 * NBLK * META_BINS], f32,
                                        tag="meta", name="meta_t")
                    lo = mkey * META_BINS * NBLK * 2
                    hi = min(2 * NBLOCKS, lo + 2 * NBLK * META_BINS)
                    nc.sync.dma_start(mt_[:, 0:hi - lo], meta[:, lo:hi])
                    meta_tiles[mkey] = mt_
                skey = r // SELF_BINS
                if skey not in self_tiles:
                    st_ = sfp.tile([128, SELF_BINS * DTILE], bf16, tag="sf",
                                   name="self_t")
                    lo = skey * SELF_BINS * DTILE
                    nc.sync.dma_start(st_[:], selfp[:, lo:lo + SELF_BINS * DTILE])
                    self_tiles[skey] = st_

            def prep(r):
                """Transpose bin r's P-path M^T columns into an edge-major
                M-tile, and kick the gathers later bins will need. Runs 2
                bins ahead of agg() so the copy latency stays off the agg
                path. stream layout per bin: [E 256 | O 256] per sub-table;
                E = even sources (parity 0), O = odd (parity 1). mtile
                blocks: 0-1 E1, 2-3 O1, 4-5 E2, 6-7 O2."""
                ensure_dcall(min(((r + 3) * BLK_D + BLK_D - 1) // DCALL_BLK,
                                 ND_CALLS - 1))
                ensure_meta(r)
                ensure_chunk_loads(min((r + 9) // BINS_PER_CHUNK, CHUNKS - 1))
                mtev, mtov = ensure_chunk(r // BINS_PER_CHUNK)
                ensure_chunk(min((r + 6) // BINS_PER_CHUNK, CHUNKS - 1))
                bb = r % BINS_PER_CHUNK
                mtile = mtilep.tile([128, 1024], bf16, tag="mtile")
                boff = bb * 2 * CAP_P
                ps_ = pstp.tile([128, 1024], bf16, tag="pst")
                for half, mv in ((0, mtev), (1, mtov)):
                    for k in range(2 * BLK_P):
                        par = 0 if k < BLK_P else 1
                        col0 = boff + k * 128
                        nc.tensor.transpose(
                            ps_[:, (half * 4 + k) * 128:(half * 4 + k + 1) * 128],
                            mv[:, col0:col0 + 128, par],
                            id_t[:],
                        )
                nc.scalar.activation(mtile[:], ps_[:],
                                     mybir.ActivationFunctionType.Copy)
                mtiles[r] = mtile

            group_state = {}

            bin_ps = {}

            def agg(r):
                ensure_meta(r)
                meta_t = meta_tiles[r // META_BINS]
                moff = (r % META_BINS) * NBLK * 2
                mtile = mtiles.pop(r)

                ps = psps.tile([128, DTILE], f32, tag="ps")
                for blk in range(NBLK):
                    s = sp.tile([128, DTILE], bf16, tag="s")
                    nc.vector.tensor_scalar(
                        out=s[:],
                        in0=iota_t[:],
                        scalar1=meta_t[:, moff + 2 * blk:moff + 2 * blk + 1],
                        scalar2=meta_t[:, moff + 2 * blk + 1:moff + 2 * blk + 2],
                        op0=mybir.AluOpType.is_equal,
                        op1=mybir.AluOpType.mult,
                    )
                    if blk < 4 * BLK_P:
                        lhsT = mtile[:, blk * 128:(blk + 1) * 128]
                    else:
                        db = r * BLK_D + (blk - 4 * BLK_P)
                        k, kb = db // DCALL_BLK, db % DCALL_BLK
                        ensure_dcall(k)
                        lhsT = dm_tiles[k][:, kb, :]
                    nc.tensor.matmul(
                        out=ps[:], lhsT=lhsT, rhs=s[:],
                        start=(blk == 0), stop=(blk == NBLK - 1),
                    )
                bin_ps[r] = ps

            def combine(r):
                """Self-add + dense chain, deferred one bin behind agg so the
                DVE stream never stalls waiting the bin's last matmul."""
                ps = bin_ps.pop(r)
                skey = r // SELF_BINS
                if r % 2 == 0:
                    group_state["s_sb"] = ssb.tile([128, 256], bf16,
                                                   tag="s_sb", name="s_sb")
                s_sb = group_state["s_sb"]
                nc.vector.tensor_tensor(
                    out=s_sb[:, (r % 2) * DTILE:(r % 2 + 1) * DTILE],
                    in0=ps[:],
                    in1=self_tiles[skey][:, (r % SELF_BINS) * DTILE:
                                         (r % SELF_BINS + 1) * DTILE],
                    op=mybir.AluOpType.add,
                )
                if r % 2 == 1:
                    q = r // 2
                    h1ps = psh.tile([128, 512], f32, tag="h1ps")
                    nc.tensor.matmul(out=h1ps[:, 0:256], lhsT=w1t[:, 0:128],
                                     rhs=s_sb[:], start=True, stop=True)
                    nc.tensor.matmul(out=h1ps[:, 256:512], lhsT=w1t[:, 128:256],
                                     rhs=s_sb[:], start=True, stop=True)
                    h1 = hsb.tile([128, 512], bf16, tag="h1")
                    nc.scalar.activation(h1[:, 0:256], h1ps[:, 0:256],
                                         mybir.ActivationFunctionType.Relu,
                                         bias=b1t[:, 0:1])
                    nc.scalar.activation(h1[:, 256:512], h1ps[:, 256:512],
                                         mybir.ActivationFunctionType.Relu,
                                         bias=b1t[:, 1:2])
                    gps = psg.tile([128, 256], f32, tag="gps")
                    nc.tensor.matmul(out=gps[:], lhsT=w2t[:, 0:128],
                                     rhs=h1[:, 0:256], start=True, stop=False)
                    nc.tensor.matmul(out=gps[:], lhsT=w2t[:, 128:256],
                                     rhs=h1[:, 256:512], start=False, stop=True)
                    gsb = gsbp.tile([128, 256], bf16, tag="gsb")
                    nc.scalar.activation(gsb[:], gps[:],
                                         mybir.ActivationFunctionType.Copy)
                    nc.sync.dma_start(gout[q], gsb[:])

            PIPE = 2
            CDEF = 2
            for r in range(R + PIPE + CDEF):
                if r >= PIPE + CDEF:
                    combine(r - PIPE - CDEF)
                if r < R:
                    prep(r)
                if PIPE <= r < R + PIPE:
                    agg(r - PIPE)

    nc.compile()
    return nc


def _get_program():
    if "p" not in _progs:
        _progs["p"] = _build_program()
    return _progs["p"]


CAPS5 = (CAP_P, CAP_P, CAP_P, CAP_P, CAP_D)


def _pack_core(deg5):
    """Greedy-balance NPC nodes into R bins with per-class caps.
    deg5: [NPC, 5] (E1, O1, E2, O2, D) degree per node. Returns bin_of or None."""
    caps = np.array(CAPS5, np.int64)
    tot = deg5.sum(axis=1)
    order = np.argsort(-tot, kind="stable")
    binload = np.zeros((R, 5), np.int64)
    bincnt = np.zeros(R, np.int64)
    bin_of = np.full(NPC, -1, np.int32)
    for v in order:
        d = deg5[v]
        nl = binload + d
        ok = (bincnt < DTILE) & (nl <= caps).all(axis=1)
        if not ok.any():
            return None
        score = (nl.astype(np.float64) / caps).max(axis=1)
        score = np.where(ok, score, np.inf)
        b = int(np.argmin(score))
        bin_of[v] = b
        bincnt[b] += 1
        binload[b] += d
    return bin_of


def _preprocess_graph(edge):
    """Per graph: class assignment, per-core packing, slot assembly."""
    src = np.asarray(edge[0], np.int64)
    dst = np.asarray(edge[1], np.int64)
    deg = np.bincount(dst, minlength=N_NODES).astype(np.float32)
    dinv = (1.0 / np.sqrt(deg + np.float32(1.0))).astype(np.float32)
    anorm = (dinv[src] * dinv[dst]).astype(np.float32)

    cores = []
    for c in range(CORES_PER_GRAPH):
        lo, hi = c * NPC, (c + 1) * NPC
        emask = (dst >= lo) & (dst < hi)
        es = src[emask]
        ed = dst[emask] - lo
        en = anorm[emask]

        # class: 0=E1, 1=O1 (src<QUART), 2=E2, 3=O2 (QUART<=src<HALF), 4=D;
        # flex zone [DWIN, HALF) demotable from E2/O2 to D
        ecls = np.where(es >= HALF, 4,
                        np.where(es < QUART, es % 2, 2 + es % 2)).astype(np.int64)
        flex = (es >= DWIN) & (es < HALF)

        for attempt in range(8):
            deg5 = np.zeros((NPC, 5), np.int64)
            np.add.at(deg5, (ed, ecls), 1)
            bin_of = _pack_core(deg5)
            if bin_of is not None:
                break
            # demote a random slice of flex edges to D and retry
            fi = np.where(flex & (ecls != 4))[0]
            rng = np.random.default_rng(attempt)
            take = fi[rng.random(len(fi)) < 0.25]
            ecls[take] = 4
        assert bin_of is not None, "bin packing failed"

        order2 = np.lexsort((np.arange(NPC), bin_of))
        pos_in_bin = np.empty(NPC, np.int64)
        binstart = np.zeros(R + 1, np.int64)
        np.cumsum(np.bincount(bin_of, minlength=R), out=binstart[1:])
        pos_in_bin[order2] = np.arange(NPC) - binstart[bin_of[order2]]

        cols_map = np.full((GROUPS, 256), -1, np.int64)
        q_of = bin_of // 2
        col_of = (bin_of % 2) * DTILE + pos_in_bin
        cols_map[q_of, col_of] = np.arange(lo, hi)
        self_rows = np.full(NSLOTS, -1, np.int64)
        self_rows[bin_of * DTILE + pos_in_bin] = np.arange(lo, hi)

        # --- slot assembly per class stream ---
        ebin = bin_of[ed]
        epos = pos_in_bin[ed]
        streams = {}
        for cls in range(5):
            cm = ecls == cls
            cap = CAPS5[cls]
            k = ebin[cm]
            okey = np.lexsort((np.arange(cm.sum()), k))
            ksorted = k[okey]
            counts = np.bincount(ksorted, minlength=R)
            assert counts.max() <= cap, (cls, counts.max())
            starts = np.zeros(R + 1, np.int64)
            np.cumsum(counts, out=starts[1:])
            within = np.arange(cm.sum()) - starts[ksorted]
            slot = ksorted * cap + within
            total = R * cap
            idx_slots = np.zeros(total, np.int64)
            dst_slots = np.full(total, -1.0, np.float32)
            nrm_slots = np.zeros(total, np.float32)
            s_src = es[cm][okey]
            if cls == 4:
                idx_slots[slot] = s_src - DWIN
            elif cls >= 2:
                idx_slots[slot] = (s_src - QUART) >> 1
            else:
                idx_slots[slot] = s_src >> 1
            dst_slots[slot] = epos[cm][okey].astype(np.float32)
            nrm_slots[slot] = en[cm][okey]
            streams[cls] = (idx_slots, dst_slots, nrm_slots)

        def wrap16(v):
            w = v.reshape(-1, 16).T  # [16, n/16]
            return np.tile(w, (8, 1)).astype(np.int16)

        def interleave(a, b):
            # per-bin [a-run | b-run]: [R, cap] + [R, cap] -> [R*2*cap]
            return np.concatenate(
                [a.reshape(R, -1), b.reshape(R, -1)], axis=1).reshape(-1)

        idxe = wrap16(interleave(streams[0][0], streams[1][0]))
        idxo = wrap16(interleave(streams[2][0], streams[3][0]))
        idxd = wrap16(streams[4][0])

        # meta: per global block (bin-major, 16 blocks: E1 E1 O1 O1 E2 E2 O2
        # O2 D0-7), cols (2b, 2b+1) = (dstrel, norm)
        dstm = np.empty((NBLOCKS, 128), np.float32)
        nrmm = np.empty((NBLOCKS, 128), np.float32)
        for cls, nblk_c, off in ((0, BLK_P, 0), (1, BLK_P, BLK_P),
                                 (2, BLK_P, 2 * BLK_P), (3, BLK_P, 3 * BLK_P),
                                 (4, BLK_D, 4 * BLK_P)):
            d2 = streams[cls][1].reshape(R, nblk_c, 128)
            n2 = streams[cls][2].reshape(R, nblk_c, 128)
            for b in range(nblk_c):
                dstm[np.arange(R) * NBLK + off + b] = d2[:, b]
                nrmm[np.arange(R) * NBLK + off + b] = n2[:, b]
        pair = np.stack([dstm, nrmm], axis=1)  # [NBLOCKS, 2, 128]
        meta = np.ascontiguousarray(pair.reshape(NBLOCKS * 2, 128).T).astype(np.float32)

        cores.append({
            "idxe": idxe, "idxo": idxo, "idxd": idxd, "meta": meta,
            "cols_map": cols_map, "self_rows": self_rows, "lo": lo,
        })
    return cores, dinv


def _self_block(core, tbl_f32, dinv):
    sr = core["self_rows"]
    valid = sr >= 0
    blk = np.zeros((NSLOTS, D_IN), np.float32)
    v = sr[valid]
    blk[valid] = tbl_f32[v] * (dinv[v] * dinv[v])[:, None]
    return np.ascontiguousarray(blk.T).astype(BF16)


def _pack_ftab(tbl_b16):
    """[128, NPAIR] int32: ftab[p, m] = bf16x2(tbl[2m, p], tbl[2m+1, p])."""
    xb = np.ascontiguousarray(tbl_b16[0:HALF]).view(np.uint16).reshape(NPAIR, 2, 128)
    packed = xb[:, 0, :].astype(np.uint32) | (xb[:, 1, :].astype(np.uint32) << 16)
    return np.ascontiguousarray(packed.T).view(np.int32)


def _assemble(results, cores_list):
    outs = []
    for g, cores in enumerate(cores_list):
        full = np.zeros((N_NODES, D_IN), np.float32)
        for c in range(CORES_PER_GRAPH):
            go = np.asarray(results[g * CORES_PER_GRAPH + c]["gout"], dtype=np.float32)
            cm = cores[c]["cols_map"]
            for q in range(GROUPS):
                valid = cm[q] >= 0
                full[cm[q][valid]] = go[q][:, valid].T
        outs.append(full)
    return outs


def _spot_check(full, tbl, edge, dinv, post, n_samples=24, tol=5e-2):
    src = np.asarray(edge[0], np.int64)
    dst = np.asarray(edge[1], np.int64)
    rng = np.random.default_rng(12345)
    nodes = rng.integers(0, N_NODES, size=n_samples)
    for v in nodes:
        ine = np.where(dst == v)[0]
        s = (dinv[src[ine]] * dinv[v])[:, None] * tbl[src[ine]]
        s = s.sum(axis=0, dtype=np.float64) + np.float64(dinv[v]) ** 2 * tbl[v]
        exp = post(s)
        got = full[v]
        scale = max(np.abs(exp).max(), 1e-3)
        if np.abs(got - exp).max() / scale > tol:
            return False
    return True


LAUNCH_WALL = []
IOTA = np.ascontiguousarray(
    np.broadcast_to(np.arange(DTILE, dtype=np.float32), (128, DTILE))).astype(BF16)
IDENT = np.eye(128, dtype=np.float32).astype(BF16)


def kernel(x_a, edge_a, x_b, edge_b, W1, b1, W2, b2):
    x_a = np.ascontiguousarray(np.asarray(x_a, np.float32))
    x_b = np.ascontiguousarray(np.asarray(x_b, np.float32))
    W1 = np.asarray(W1, np.float32)
    b1 = np.asarray(b1, np.float32)
    W2 = np.asarray(W2, np.float32)
    b2 = np.asarray(b2, np.float32)

    nc = _get_program()
    cores_a, dinv_a = _preprocess_graph(np.asarray(edge_a))
    cores_b, dinv_b = _preprocess_graph(np.asarray(edge_b))

    b1c = np.stack([b1[0:128], b1[128:256]], axis=1).astype(np.float32)
    eye = np.eye(128, dtype=np.float32)
    w1_id = np.concatenate([eye, np.zeros((128, 128), np.float32)], axis=1).astype(BF16)
    w2_id = np.concatenate([eye, np.zeros((128, 128), np.float32)], axis=0).astype(BF16)
    b1c_id = np.stack([b2, np.zeros(128, np.float32)], axis=1).astype(np.float32)
    w1_b = W1.astype(BF16)
    w2_b = W2.astype(BF16)

    def maps(tbl_a, tbl_b, w1m, w2m, b1m):
        ms = []
        for tf, cores, dinv in ((tbl_a, cores_a, dinv_a), (tbl_b, cores_b, dinv_b)):
            tb = tf.astype(BF16)
            ft = _pack_ftab(tb)
            for c in range(CORES_PER_GRAPH):
                ms.append({
                    "tbl": tb, "ftab": ft,
                    "idxe": cores[c]["idxe"], "idxo": cores[c]["idxo"],
                    "idxd": cores[c]["idxd"], "meta": cores[c]["meta"],
                    "iota": IOTA, "ident": IDENT,
                    "selfp": _self_block(cores[c], tf, dinv),
                    "w1": w1m, "w2": w2m, "b1c": b1m,
                })
        return ms

    core_ids = list(range(N_CORES))

    def run(in_maps):
        import time as _t
        last = None
        for attempt in range(4):
            try:
                t0 = _t.time()
                res = run_bass_kernel_spmd(nc, in_maps, core_ids)
                LAUNCH_WALL.append(_t.time() - t0)
                return res
            except Exception as e:
                last = e
                _t.sleep(5)
        raise last

    def post_a(s):
        return np.maximum(s @ W1.astype(np.float64) + b1, 0.0) @ W2.astype(np.float64)

    def post_b(s):
        return np.maximum(s + b2, 0.0)

    for attempt in range(4):
        resA = run(maps(x_a, x_b, w1_b, w2_b, b1c))
        g_a, g_b = _assemble(resA.results, (cores_a, cores_b))
        if (_spot_check(g_a, x_a, edge_a, dinv_a, post_a)
                and _spot_check(g_b, x_b, edge_b, dinv_b, post_a)):
            break
    for attempt in range(4):
        resB = run(maps(g_a, g_b, w1_id, w2_id, b1c_id))
        z_a, z_b = _assemble(resB.results, (cores_a, cores_b))
        if (_spot_check(z_a, g_a, edge_a, dinv_a, post_b)
                and _spot_check(z_b, g_b, edge_b, dinv_b, post_b)):
            break
    return (z_a, z_b)
